# revision 8
# baseline (speedup 1.0000x reference)
"""GAT (2-layer, 4-head) message-passing kernel for 8 Trainium2 NeuronCores.

Sharding: nodes split into 8 contiguous ranges of 6250 (padded to 6272); within
each core nodes are sorted by in-degree into 49 windows of 128 (one dst node
per SBUF partition). Each core builds hidden-table rows (h | a_s | a_d) for its
nodes, the table is AllGathered, and each core processes its own in-edges:
edge slot (p, c) = c-th in-edge of the window's p-th node. h[src] rows are
fetched with dma_gather using int16 PAIR row indices (2x320 f32 = 2560B
descriptors); a parity mask zeroes the unused pair half. Per-edge softmax
weights ex = exp(leakyrelu(a_s[src]+a_d[dst])) multiply the messages on DVE,
and identity-weight matmuls accumulate the per-partition sums in PSUM (with ex
riding along as 4 extra columns -> softmax denominators). Normalization, head
mean, batchnorm moments (ones-matmuls + 2xC AllReduce) and the MLP head follow.
Biases b1/b2 cancel inside the following batchnorms and are dropped.
"""

import numpy as np

N = 50000
E = 800000
IN = 128
T = 8
H = 4
F = 64
C = 256
END = 256
NCORES = 8
NPC = 6250
NPCP = 6272
NW = NPCP // 128
P = 128
DW = 288              # table row: 256 h | 4 a_s | 4 a_d | 24 pad
ROWS = NCORES * NPCP
EPS = 1e-5
CAP = 10              # max slot-columns per gather chunk
COMBINE = True        # pre-combine parity slabs on Pool before the PE matmul

_CACHE = {}


def _host_prep(X, edge_index):
    ei = np.asarray(edge_index)
    src = ei[0].astype(np.int64)
    dst = ei[1].astype(np.int64)
    deg = np.bincount(dst, minlength=N)

    # global degree-desc ranking: rank r -> window r//1024, core (r%1024)//128,
    # lane r%128.  All cores share each window's degree profile, so the shared
    # per-window max (CW) tracks the true degrees tightly.
    order = np.argsort(-deg, kind="stable")
    perm = np.empty(NCORES * NPCP, np.int64)
    perm.fill(-1)
    tpos = np.empty(N, np.int64)
    r = np.arange(N)
    w_of = r // (NCORES * P)
    core_of = (r % (NCORES * P)) // P
    lane_of = r % P
    pos = core_of * NPCP + w_of * P + lane_of
    perm[pos] = order
    tpos[order] = pos

    stp = tpos[src]
    dtp = tpos[dst]
    dcore = dtp // NPCP
    dlocal = dtp % NPCP

    degs = np.zeros(NCORES * NPCP, np.int64)
    degs[tpos[np.arange(N)]] = deg
    cw = degs.reshape(NCORES, NW, P).max(axis=2)
    CW = [int(x) for x in np.maximum(cw.max(axis=0), 1)]
    woff = np.concatenate([[0], np.cumsum(np.array(CW, np.int64))])
    slots = int(woff[-1]) * P

    order = np.lexsort((stp, dtp))
    sdtp, sstp = dtp[order], stp[order]
    sdcore, sdlocal = dcore[order], dlocal[order]
    uniq, counts = np.unique(sdtp, return_counts=True)
    ranks = np.arange(E) - np.repeat(np.cumsum(counts) - counts, counts)

    w = sdlocal // P
    p = sdlocal % P
    slot = (woff[w] + ranks) * P + p

    idx_pair = np.zeros((NCORES, slots), np.int16)
    pmask = np.zeros((NCORES, slots, 2), np.float32)
    for c in range(NCORES):
        m = sdcore == c
        sl = slot[m]
        st = sstp[m]
        idx_pair[c, sl] = (st // 2).astype(np.int16)
        pmask[c, sl, 0] = (st % 2 == 0).astype(np.float32)
        pmask[c, sl, 1] = (st % 2 == 1).astype(np.float32)

    def pack16(a):
        b = a.reshape(-1, 16).T
        return np.tile(b, (8, 1))

    idx_tiles = np.stack([pack16(idx_pair[c]) for c in range(NCORES)])
    pm = pmask.reshape(NCORES, slots // P, P, 2).transpose(0, 2, 1, 3).copy()

    Xf = np.ascontiguousarray(X[:, :, T - 1]).astype(np.float32)
    xrows = np.zeros((NCORES, NPCP, IN), np.float32)
    for c in range(NCORES):
        pc = perm[c * NPCP : (c + 1) * NPCP]
        m = pc >= 0
        xrows[c, m] = Xf[pc[m]]

    return dict(CW=CW, woff=woff, slots=slots, idx_tiles=idx_tiles,
                pm=pm, perm=perm, xrows=xrows)


def _build_weights(inp):
    f32 = np.float32
    W_in = np.asarray(inp["W_in"], f32)
    W1 = np.asarray(inp["W1"], f32)
    W2 = np.asarray(inp["W2"], f32)

    def att_mat(a_s, a_d):
        A = np.zeros((C, 2 * H), f32)
        for k in range(H):
            A[64 * k : 64 * (k + 1), k] = a_s[k]
            A[64 * k : 64 * (k + 1), H + k] = a_d[k]
        return A

    WA1 = W1 @ att_mat(np.asarray(inp["as1"], f32), np.asarray(inp["ad1"], f32))
    WA2 = W2 @ att_mat(np.asarray(inp["as2"], f32), np.asarray(inp["ad2"], f32))
    b_in = np.asarray(inp["b_in"], f32)
    return dict(
        W_in=W_in,
        W1s=np.ascontiguousarray(np.stack([W1[:128], W1[128:]], axis=1)),
        W2s=np.ascontiguousarray(np.stack([W2[:128], W2[128:]], axis=1)),
        WA1s=np.ascontiguousarray(np.stack([WA1[:128], WA1[128:]], axis=1)),
        WA2s=np.ascontiguousarray(np.stack([WA2[:128], WA2[128:]], axis=1)),
        b_in_cols=np.ascontiguousarray(np.stack([b_in[:128], b_in[128:]], 1)),
        g1=np.asarray(inp["g1"], f32)[None, :],
        be1=np.asarray(inp["be1"], f32)[None, :],
        g2=np.asarray(inp["g2"], f32)[None, :],
        be2=np.asarray(inp["be2"], f32)[None, :],
        Wo1=np.asarray(inp["Wo1"], f32),
        bo1=np.asarray(inp["bo1"], f32)[None, :],
        Wo2rep=np.ascontiguousarray(
            np.broadcast_to(np.asarray(inp["Wo2"], f32)[:, 0][None, :], (P, C))),
        bo2rep=np.full((P, 1), float(np.asarray(inp["bo2"]).reshape(-1)[0]), f32),
        ident=np.eye(P, dtype=f32),
        ones=np.ones((P, 1), f32),
        ones_row=np.ones((1, P), f32),
    )


def _build_program(CW, woff, slots, repeat=1, local_cc=None):
    import concourse.bacc as bacc
    import concourse.tile as tile
    from concourse import mybir

    if local_cc is None:
        local_cc = repeat > 1
    nc = bacc.Bacc("TRN2", num_devices=NCORES)
    dt = mybir.dt
    f32 = dt.float32
    AX = mybir.AxisListType
    OP = mybir.AluOpType
    ACT = mybir.ActivationFunctionType
    CCG = [list(range(NCORES))]

    d_x = nc.declare_dram_parameter("xrows", [NPCP, IN], f32, isOutput=False)
    d_idx = nc.declare_dram_parameter("idx_tiles", [P, slots // 16], dt.int16,
                                      isOutput=False)
    d_pm = nc.declare_dram_parameter("pm", [P, slots // P, 2], f32, isOutput=False)
    d_Win = nc.declare_dram_parameter("W_in", [IN, C], f32, isOutput=False)
    d_W1s = nc.declare_dram_parameter("W1s", [P, 2, C], f32, isOutput=False)
    d_W2s = nc.declare_dram_parameter("W2s", [P, 2, C], f32, isOutput=False)
    d_WA1s = nc.declare_dram_parameter("WA1s", [P, 2, 2 * H], f32, isOutput=False)
    d_WA2s = nc.declare_dram_parameter("WA2s", [P, 2, 2 * H], f32, isOutput=False)
    d_binc = nc.declare_dram_parameter("b_in_cols", [P, 2], f32, isOutput=False)
    d_g1 = nc.declare_dram_parameter("g1", [1, C], f32, isOutput=False)
    d_be1 = nc.declare_dram_parameter("be1", [1, C], f32, isOutput=False)
    d_g2 = nc.declare_dram_parameter("g2", [1, F], f32, isOutput=False)
    d_be2 = nc.declare_dram_parameter("be2", [1, F], f32, isOutput=False)
    d_Wo1 = nc.declare_dram_parameter("Wo1", [F, END], f32, isOutput=False)
    d_bo1 = nc.declare_dram_parameter("bo1", [1, END], f32, isOutput=False)
    d_Wo2r = nc.declare_dram_parameter("Wo2rep", [P, C], f32, isOutput=False)
    d_bo2r = nc.declare_dram_parameter("bo2rep", [P, 1], f32, isOutput=False)
    d_id = nc.declare_dram_parameter("ident", [P, P], f32, isOutput=False)
    d_ones = nc.declare_dram_parameter("ones", [P, 1], f32, isOutput=False)
    d_onesr = nc.declare_dram_parameter("ones_row", [1, P], f32, isOutput=False)
    d_out = nc.declare_dram_parameter("out", [NPCP, 1], f32, isOutput=True)

    loc1 = nc.dram_tensor("loc1", [NPCP, DW], f32)
    tab1 = nc.dram_tensor("tab1", [ROWS, DW], f32, addr_space="Shared")
    g1loc = nc.dram_tensor("g1loc", [NPCP, C], f32)
    loc2 = nc.dram_tensor("loc2", [NPCP, DW], f32)
    tab2 = nc.dram_tensor("tab2", [ROWS, DW], f32, addr_space="Shared")
    g2loc = nc.dram_tensor("g2loc", [NPCP, F], f32)
    st1 = nc.dram_tensor("st1", [2, C], f32)
    st1r = nc.dram_tensor("st1r", [2, C], f32, addr_space="Shared")
    st2 = nc.dram_tensor("st2", [2, F], f32)
    st2r = nc.dram_tensor("st2r", [2, F], f32, addr_space="Shared")
    sc1 = nc.dram_tensor("sc1", [2, C], f32)
    sc2 = nc.dram_tensor("sc2", [2, F], f32)

    def mm(out, lhsT, rhs, start, stop):
        nc.tensor.matmul(out=out, lhsT=lhsT, rhs=rhs, start=start, stop=stop)

    import contextlib
    with tile.TileContext(nc) as tc:
        with (
            tc.tile_pool(name="const", bufs=1) as cpool,
            tc.tile_pool(name="sbuf", bufs=2) as sbuf,
            tc.tile_pool(name="gat", bufs=2) as gpool,
            tc.tile_pool(name="msgp", bufs=2) as mpool,
            tc.tile_pool(name="psum", bufs=2, space="PSUM") as psum,
            tc.tile_pool(name="pstat", bufs=1, space="PSUM") as pstat,
        ):
            def ctile(dram, shape, tag, dtt=f32):
                t = cpool.tile(shape, dtt, tag=tag)
                nc.sync.dma_start(out=t[:], in_=dram[:])
                return t

            ident = ctile(d_id, [P, P], "ident")
            ones = ctile(d_ones, [P, 1], "ones")
            ones_r2 = cpool.tile([P, P], f32, tag="ones_r")
            nc.sync.dma_start(out=ones_r2[0:1, :], in_=d_onesr[:])
            Win_t = ctile(d_Win, [IN, C], "Win")
            W1_t = ctile(d_W1s, [P, 2, C], "W1")
            W2_t = ctile(d_W2s, [P, 2, C], "W2")
            WA1_t = ctile(d_WA1s, [P, 2, 2 * H], "WA1")
            WA2_t = ctile(d_WA2s, [P, 2, 2 * H], "WA2")
            binc_t = ctile(d_binc, [P, 2], "binc")
            Wo1_t = cpool.tile([P, END], f32, tag="Wo1")
            nc.sync.dma_start(out=Wo1_t[0:F, :], in_=d_Wo1[:])
            bo1_t = cpool.tile([P, END], f32, tag="bo1")
            nc.sync.dma_start(out=bo1_t[0:1, :], in_=d_bo1[:])
            Wo2r_t = ctile(d_Wo2r, [P, C], "Wo2r")
            bo2r_t = ctile(d_bo2r, [P, 1], "bo2r")
            idx_t = ctile(d_idx, [P, slots // 16], "idxt", dt.int16)
            pm_t = ctile(d_pm, [P, slots // P, 2], "pmt")

            rep_cm = tc.For_i(0, repeat, 1) if repeat > 1 else contextlib.nullcontext()
            with rep_cm:
                # ---------------- table-row builder -------------------------
                def build_table(rows_getter, W_t, WA_t, loc):
                    for t in range(NW):
                        yT = rows_getter(t)
                        ph = psum.tile([P, C + H], f32, space="PSUM", tag="big")
                        pa = psum.tile([P, 2 * H], f32, space="PSUM", tag="small")
                        for hf in range(2):
                            mm(ph[:, 0:C], yT[hf][:], W_t[:, hf, :],
                               start=(hf == 0), stop=(hf == 1))
                            mm(pa[:], yT[hf][:], WA_t[:, hf, :],
                               start=(hf == 0), stop=(hf == 1))
                        stg = sbuf.tile([P, DW], f32, tag="stgA")
                        nc.vector.tensor_copy(out=stg[:, 0:C], in_=ph[:, 0:C])
                        nc.vector.tensor_copy(out=stg[:, C : C + 2 * H], in_=pa[:])
                        nc.vector.memset(stg[:, C + 2 * H : DW], 0.0)
                        nc.sync.dma_start(out=loc[t * P : (t + 1) * P, :], in_=stg[:])

                # ---------------- phase A ------------------------------------
                def phaseA_rows(t):
                    xs = sbuf.tile([P, IN], f32, tag="xs")
                    nc.sync.dma_start(out=xs[:], in_=d_x[t * P : (t + 1) * P, :])
                    pt = psum.tile([P, P], f32, space="PSUM", tag="tr")
                    nc.tensor.transpose(out=pt[:], in_=xs[:], identity=ident[:])
                    xsT = sbuf.tile([P, P], f32, tag="xsT")
                    nc.vector.tensor_copy(out=xsT[:], in_=pt[:])
                    yT = []
                    for hf in range(2):
                        px = psum.tile([P, P], f32, space="PSUM", tag="tr")
                        mm(px[:], Win_t[:, hf * P : (hf + 1) * P], xsT[:],
                           start=True, stop=True)
                        xt = sbuf.tile([P, P], f32, tag=f"x0T{hf}")
                        nc.vector.tensor_tensor(
                            out=xt[:], in0=px[:],
                            in1=binc_t[:, hf : hf + 1].broadcast_to([P, P]),
                            op=OP.add)
                        yT.append(xt)
                    return yT

                build_table(phaseA_rows, W1_t, WA1_t, loc1)
                if local_cc:
                    nc.sync.dma_start(out=tab1[0:NPCP, :], in_=loc1[:])
                else:
                    nc.gpsimd.collective_compute(
                        "AllGather", OP.bypass, replica_groups=CCG,
                        ins=[loc1[:].opt()], outs=[tab1[:].opt()])

                # ---------------- edge phase ---------------------------------
                def edge_phase(tab, loc, layer):
                    outw = C if layer == 1 else F
                    pstats = pstat.tile([P, C], f32, space="PSUM", tag="sx")
                    pstats2 = pstat.tile([P, C], f32, space="PSUM", tag="sxx")
                    tabv = tab[:].rearrange("(q two) d -> q (two d)", two=2)
                    for w in range(NW):
                        cw = CW[w]
                        off = int(woff[w])
                        attD = sbuf.tile([P, H], f32, tag="attD")
                        nc.sync.dma_start(
                            out=attD[:],
                            in_=loc[w * P : (w + 1) * P, C + H : C + 2 * H])
                        po = psum.tile([P, C + H], f32, space="PSUM", tag="big")
                        nsub = (cw + CAP - 1) // CAP
                        szs = [cw // nsub + (1 if i < cw % nsub else 0)
                               for i in range(nsub)]
                        offs = [sum(szs[:i]) for i in range(nsub)]
                        for s in range(nsub):
                            c0 = offs[s]
                            ns = szs[s]
                            hg = gpool.tile([P, CAP, 2 * DW], f32, tag="hg")
                            nc.gpsimd.dma_gather(
                                out_ap=hg[:, 0:ns, :],
                                in_ap=tabv,
                                idxs_ap=idx_t[:, (off + c0) * 8 : (off + c0 + ns) * 8],
                                num_idxs=ns * P,
                                num_idxs_reg=ns * P,
                                elem_size=2 * DW,
                                single_packet=False,
                            )
                            hgv = hg[:, 0:ns, :].rearrange(
                                "p c (two d) -> p c two d", two=2)
                            ex = mpool.tile([P, CAP, 2, H], f32, tag="ex")
                            nc.vector.tensor_tensor(
                                out=ex[:, 0:ns],
                                in0=hgv[:, :, :, C : C + H],
                                in1=attD[:].unsqueeze(1).unsqueeze(1)
                                    .broadcast_to([P, ns, 2, H]),
                                op=OP.add)
                            lr = mpool.tile([P, CAP, 2, H], f32, tag="lr")
                            nc.vector.tensor_scalar(
                                out=lr[:, 0:ns], in0=ex[:, 0:ns], scalar1=0.2,
                                scalar2=None, op0=OP.mult)
                            nc.vector.tensor_tensor(
                                out=lr[:, 0:ns], in0=lr[:, 0:ns], in1=ex[:, 0:ns],
                                op=OP.max)
                            nc.scalar.activation(out=ex[:, 0:ns], in_=lr[:, 0:ns],
                                                 func=ACT.Exp)
                            nc.vector.tensor_tensor(
                                out=ex[:, 0:ns], in0=ex[:, 0:ns],
                                in1=pm_t[:, off + c0 : off + c0 + ns, :]
                                    .unsqueeze(3).broadcast_to([P, ns, 2, H]),
                                op=OP.mult)
                            for par in range(2):
                                nc.vector.tensor_tensor(
                                    out=hgv[:, :, par, 0:C].rearrange(
                                        "p c (k f) -> p c k f", k=H),
                                    in0=hgv[:, :, par, 0:C].rearrange(
                                        "p c (k f) -> p c k f", k=H),
                                    in1=ex[:, 0:ns, par, :].unsqueeze(3)
                                        .broadcast_to([P, ns, H, F]),
                                    op=OP.mult)
                            nc.vector.tensor_copy(
                                out=hgv[:, :, :, C : C + H], in_=ex[:, 0:ns])
                            if COMBINE:
                                msgc = mpool.tile([P, CAP, C + H], f32, tag="msgc")
                                nc.vector.tensor_tensor(
                                    out=msgc[:, 0:ns],
                                    in0=hgv[:, :, 0, 0 : C + H],
                                    in1=hgv[:, :, 1, 0 : C + H], op=OP.add)
                                for cc in range(ns):
                                    mm(po[:], ident[:], msgc[:, cc, :],
                                       start=(s == 0 and cc == 0),
                                       stop=(s == nsub - 1 and cc == ns - 1))
                            else:
                                for cc in range(ns):
                                    for par in range(2):
                                        mm(po[:], ident[:],
                                           hgv[:, cc, par, 0 : C + H],
                                           start=(s == 0 and cc == 0 and par == 0),
                                           stop=(s == nsub - 1 and cc == ns - 1
                                                 and par == 1))
                        # flush
                        sden = sbuf.tile([P, H], f32, tag="sden")
                        nc.vector.tensor_scalar(out=sden[:], in0=po[:, C : C + H],
                                                scalar1=1e-16, scalar2=None,
                                                op0=OP.add)
                        rs = sbuf.tile([P, H], f32, tag="rs")
                        nc.vector.reciprocal(out=rs[:], in_=sden[:])
                        if layer == 1:
                            org = sbuf.tile([P, C], f32, tag="org")
                            nc.vector.tensor_tensor(
                                out=org[:].rearrange("p (k f) -> p k f", k=H),
                                in0=po[:, 0:C].rearrange("p (k f) -> p k f", k=H),
                                in1=rs[:].unsqueeze(2).broadcast_to([P, H, F]),
                                op=OP.mult)
                            nc.sync.dma_start(out=g1loc[w * P : (w + 1) * P, :],
                                              in_=org[:])
                        else:
                            nc.vector.tensor_scalar(out=rs[:], in0=rs[:],
                                                    scalar1=0.25, scalar2=None,
                                                    op0=OP.mult)
                            tmp = sbuf.tile([P, C], f32, tag="tmp2")
                            nc.vector.tensor_tensor(
                                out=tmp[:].rearrange("p (k f) -> p k f", k=H),
                                in0=po[:, 0:C].rearrange("p (k f) -> p k f", k=H),
                                in1=rs[:].unsqueeze(2).broadcast_to([P, H, F]),
                                op=OP.mult)
                            org = sbuf.tile([P, F], f32, tag="orgf")
                            nc.vector.tensor_tensor(out=org[:], in0=tmp[:, 0:F],
                                                    in1=tmp[:, F : 2 * F], op=OP.add)
                            nc.vector.tensor_tensor(out=org[:], in0=org[:],
                                                    in1=tmp[:, 2 * F : 3 * F],
                                                    op=OP.add)
                            nc.vector.tensor_tensor(out=org[:], in0=org[:],
                                                    in1=tmp[:, 3 * F : 4 * F],
                                                    op=OP.add)
                            nc.sync.dma_start(out=g2loc[w * P : (w + 1) * P, :],
                                              in_=org[:])
                        sq = sbuf.tile([P, C], f32, tag="sq")
                        nc.vector.tensor_tensor(out=sq[:, 0:outw], in0=org[:],
                                                in1=org[:], op=OP.mult)
                        mm(pstats[0:1, 0:outw], ones[:], org[:],
                           start=(w == 0), stop=(w == NW - 1))
                        mm(pstats2[0:1, 0:outw], ones[:], sq[:, 0:outw],
                           start=(w == 0), stop=(w == NW - 1))
                    # moments -> AllReduce -> scale/shift rows in DRAM
                    stg0 = sbuf.tile([P, C], f32, tag="stg0")
                    nc.vector.tensor_copy(out=stg0[0:1, 0:outw],
                                          in_=pstats[0:1, 0:outw])
                    stg1 = sbuf.tile([P, C], f32, tag="stg1")
                    nc.vector.tensor_copy(out=stg1[0:1, 0:outw],
                                          in_=pstats2[0:1, 0:outw])
                    std = st1 if layer == 1 else st2
                    stdr = st1r if layer == 1 else st2r
                    nc.sync.dma_start(out=std[0:1, :], in_=stg0[0:1, 0:outw])
                    nc.sync.dma_start(out=std[1:2, :], in_=stg1[0:1, 0:outw])
                    if local_cc:
                        nc.sync.dma_start(out=stdr[:, :], in_=std[:])
                    else:
                        nc.gpsimd.collective_compute(
                            "AllReduce", OP.add, replica_groups=CCG,
                            ins=[std[:].opt()], outs=[stdr[:].opt()])
                    # single-partition workspace: slices share one partition
                    bn = cpool.tile([1, 10 * C], f32, tag="bn")
                    r0 = bn[:, 0 * C : 0 * C + outw]
                    r1 = bn[:, 1 * C : 1 * C + outw]
                    gv = bn[:, 2 * C : 2 * C + outw]
                    bev = bn[:, 3 * C : 3 * C + outw]
                    mu = bn[:, 4 * C : 4 * C + outw]
                    var = bn[:, 5 * C : 5 * C + outw]
                    msq = bn[:, 6 * C : 6 * C + outw]
                    rstd = bn[:, 7 * C : 7 * C + outw]
                    scl = bn[:, 8 * C : 8 * C + outw]
                    shf = bn[:, 9 * C : 9 * C + outw]
                    nc.sync.dma_start(out=r0, in_=stdr[0:1, :])
                    nc.sync.dma_start(out=r1, in_=stdr[1:2, :])
                    nc.sync.dma_start(out=gv, in_=(d_g1 if layer == 1 else d_g2)[:])
                    nc.sync.dma_start(out=bev, in_=(d_be1 if layer == 1 else d_be2)[:])
                    nc.vector.tensor_scalar(out=mu, in0=r0, scalar1=1.0 / N,
                                            scalar2=None, op0=OP.mult)
                    nc.vector.tensor_scalar(out=var, in0=r1, scalar1=1.0 / N,
                                            scalar2=None, op0=OP.mult)
                    nc.vector.tensor_tensor(out=msq, in0=mu, in1=mu, op=OP.mult)
                    nc.vector.tensor_tensor(out=var, in0=var, in1=msq, op=OP.subtract)
                    nc.vector.tensor_scalar(out=var, in0=var, scalar1=EPS,
                                            scalar2=None, op0=OP.add)
                    nc.scalar.activation(out=msq, in_=var, func=ACT.Sqrt)
                    nc.vector.reciprocal(out=rstd, in_=msq)
                    nc.vector.tensor_tensor(out=scl, in0=gv, in1=rstd, op=OP.mult)
                    nc.vector.tensor_tensor(out=shf, in0=mu, in1=scl, op=OP.mult)
                    nc.vector.tensor_tensor(out=shf, in0=bev, in1=shf, op=OP.subtract)
                    scd = sc1 if layer == 1 else sc2
                    nc.sync.dma_start(out=scd[0:1, :], in_=scl)
                    nc.sync.dma_start(out=scd[1:2, :], in_=shf)

                edge_phase(tab1, loc1, 1)

                # ---------------- phase E ------------------------------------
                sccol1 = sbuf.tile([P, 4], f32, tag="sccol1")
                nc.sync.dma_start(
                    out=sccol1[:].rearrange("p (r h) -> p r h", r=2),
                    in_=sc1[:].rearrange("r (h p) -> p r h", p=P))

                def phaseE_rows(t):
                    g1r = sbuf.tile([P, C], f32, tag="g1r")
                    nc.sync.dma_start(out=g1r[:], in_=g1loc[t * P : (t + 1) * P, :])
                    yT = []
                    for hf in range(2):
                        ptt = psum.tile([P, P], f32, space="PSUM", tag="tr")
                        nc.tensor.transpose(out=ptt[:],
                                            in_=g1r[:, hf * P : (hf + 1) * P],
                                            identity=ident[:])
                        yt = sbuf.tile([P, P], f32, tag=f"yT{hf}")
                        nc.vector.tensor_scalar(
                            out=yt[:], in0=ptt[:],
                            scalar1=sccol1[:, hf : hf + 1],
                            scalar2=sccol1[:, 2 + hf : 3 + hf],
                            op0=OP.mult, op1=OP.add)
                        nc.vector.tensor_scalar(out=yt[:], in0=yt[:], scalar1=0.0,
                                                scalar2=None, op0=OP.max)
                        yT.append(yt)
                    return yT

                build_table(phaseE_rows, W2_t, WA2_t, loc2)
                if local_cc:
                    nc.sync.dma_start(out=tab2[0:NPCP, :], in_=loc2[:])
                else:
                    nc.gpsimd.collective_compute(
                        "AllGather", OP.bypass, replica_groups=CCG,
                        ins=[loc2[:].opt()], outs=[tab2[:].opt()])

                edge_phase(tab2, loc2, 2)

                # ---------------- phase I ------------------------------------
                sccol2 = sbuf.tile([P, 2], f32, tag="sccol2")
                nc.sync.dma_start(out=sccol2[0:F, :],
                                  in_=sc2[:].rearrange("r f -> f r"))
                for t in range(NW):
                    g2r = sbuf.tile([P, F], f32, tag="g2r")
                    nc.sync.dma_start(out=g2r[:], in_=g2loc[t * P : (t + 1) * P, :])
                    ptt = psum.tile([P, P], f32, space="PSUM", tag="tr")
                    nc.tensor.transpose(out=ptt[0:F, :], in_=g2r[:],
                                        identity=ident[:])
                    y2T = sbuf.tile([P, P], f32, tag="y2T")
                    nc.vector.tensor_scalar(
                        out=y2T[0:F, :], in0=ptt[0:F, :],
                        scalar1=sccol2[0:F, 0:1], scalar2=sccol2[0:F, 1:2],
                        op0=OP.mult, op1=OP.add)
                    pz = psum.tile([P, END], f32, space="PSUM", tag="big")
                    mm(pz[:], y2T[0:F, :], Wo1_t[0:F, :], start=True, stop=False)
                    mm(pz[:], ones_r2[0:1, :], bo1_t[0:1, :], start=False, stop=True)
                    zr = sbuf.tile([P, END], f32, tag="zr")
                    nc.vector.tensor_scalar(out=zr[:], in0=pz[:], scalar1=0.0,
                                            scalar2=None, op0=OP.max)
                    zw = sbuf.tile([P, C], f32, tag="zw")
                    nc.vector.tensor_tensor(out=zw[:], in0=zr[:], in1=Wo2r_t[:],
                                            op=OP.mult)
                    res = sbuf.tile([P, 1], f32, tag="res")
                    nc.vector.tensor_reduce(out=res[:], in_=zw[:], axis=AX.X,
                                            op=OP.add)
                    nc.vector.tensor_tensor(out=res[:], in0=res[:], in1=bo2r_t[:],
                                            op=OP.add)
                    nc.sync.dma_start(out=d_out[t * P : (t + 1) * P, :], in_=res[:])

    nc.compile()
    return nc


def kernel(**inputs):
    X = np.asarray(inputs["X"], np.float32)
    prep = _host_prep(X, inputs["edge_index"])
    wts = _build_weights(inputs)

    key = ("prog", tuple(prep["CW"]))
    if key not in _CACHE:
        _CACHE.clear()
        _CACHE[key] = _build_program(prep["CW"], prep["woff"], prep["slots"])
    nc = _CACHE[key]

    in_maps = []
    for c in range(NCORES):
        m = dict(
            xrows=prep["xrows"][c],
            idx_tiles=prep["idx_tiles"][c],
            pm=prep["pm"][c],
        )
        m.update(wts)
        in_maps.append(m)

    from concourse.bass_utils import run_bass_kernel_spmd
    res = run_bass_kernel_spmd(nc, in_maps, list(range(NCORES)))

    out = np.zeros((N, 1), np.float32)
    for c in range(NCORES):
        pc = prep["perm"][c * NPCP : (c + 1) * NPCP]
        m = pc >= 0
        out[pc[m]] = res.results[c]["out"][m, :]
    return out



# revision 9
# speedup vs baseline: 1.2907x; 1.2907x over previous
"""GAT (2-layer, 4-head) message-passing kernel for 8 Trainium2 NeuronCores.

Sharding: nodes split into 8 contiguous ranges of 6250 (padded to 6272); within
each core nodes are sorted by in-degree into 49 windows of 128 (one dst node
per SBUF partition). Each core builds hidden-table rows (h | a_s | a_d) for its
nodes, the table is AllGathered, and each core processes its own in-edges:
edge slot (p, c) = c-th in-edge of the window's p-th node. h[src] rows are
fetched with dma_gather using int16 PAIR row indices (2x320 f32 = 2560B
descriptors); a parity mask zeroes the unused pair half. Per-edge softmax
weights ex = exp(leakyrelu(a_s[src]+a_d[dst])) multiply the messages on DVE,
and identity-weight matmuls accumulate the per-partition sums in PSUM (with ex
riding along as 4 extra columns -> softmax denominators). Normalization, head
mean, batchnorm moments (ones-matmuls + 2xC AllReduce) and the MLP head follow.
Biases b1/b2 cancel inside the following batchnorms and are dropped.
"""

import numpy as np

N = 50000
E = 800000
IN = 128
T = 8
H = 4
F = 64
C = 256
END = 256
NCORES = 8
NPC = 6250
NPCP = 6272
NW = NPCP // 128
P = 128
DW = 288              # table row: 256 h | 4 a_s | 4 a_d | 24 pad
ROWS = NCORES * NPCP
EPS = 1e-5
CAP = 10              # max slot-columns per gather chunk
COMBINE = True        # pre-combine parity slabs on Pool before the PE matmul

_CACHE = {}


def _host_prep(X, edge_index):
    ei = np.asarray(edge_index)
    src = ei[0].astype(np.int64)
    dst = ei[1].astype(np.int64)
    deg = np.bincount(dst, minlength=N)

    # global degree-desc ranking: rank r -> window r//1024, core (r%1024)//128,
    # lane r%128.  All cores share each window's degree profile, so the shared
    # per-window max (CW) tracks the true degrees tightly.
    order = np.argsort(-deg, kind="stable")
    perm = np.empty(NCORES * NPCP, np.int64)
    perm.fill(-1)
    tpos = np.empty(N, np.int64)
    r = np.arange(N)
    w_of = r // (NCORES * P)
    core_of = (r % (NCORES * P)) // P
    lane_of = r % P
    pos = core_of * NPCP + w_of * P + lane_of
    perm[pos] = order
    tpos[order] = pos

    stp = tpos[src]
    dtp = tpos[dst]
    dcore = dtp // NPCP
    dlocal = dtp % NPCP

    degs = np.zeros(NCORES * NPCP, np.int64)
    degs[tpos[np.arange(N)]] = deg
    cw = degs.reshape(NCORES, NW, P).max(axis=2)
    CW = [int(x) for x in np.maximum(cw.max(axis=0), 1)]
    woff = np.concatenate([[0], np.cumsum(np.array(CW, np.int64))])
    slots = int(woff[-1]) * P

    order = np.lexsort((stp, dtp))
    sdtp, sstp = dtp[order], stp[order]
    sdcore, sdlocal = dcore[order], dlocal[order]
    uniq, counts = np.unique(sdtp, return_counts=True)
    ranks = np.arange(E) - np.repeat(np.cumsum(counts) - counts, counts)

    w = sdlocal // P
    p = sdlocal % P
    slot = (woff[w] + ranks) * P + p

    idx_pair = np.zeros((NCORES, slots), np.int16)
    pmask = np.zeros((NCORES, slots, 2), np.float32)
    for c in range(NCORES):
        m = sdcore == c
        sl = slot[m]
        st = sstp[m]
        idx_pair[c, sl] = (st // 2).astype(np.int16)
        pmask[c, sl, 0] = (st % 2 == 0).astype(np.float32)
        pmask[c, sl, 1] = (st % 2 == 1).astype(np.float32)

    def pack16(a):
        b = a.reshape(-1, 16).T
        return np.tile(b, (8, 1))

    idx_tiles = np.stack([pack16(idx_pair[c]) for c in range(NCORES)])
    pm = pmask.reshape(NCORES, slots // P, P, 2).transpose(0, 2, 1, 3).copy()

    Xf = np.ascontiguousarray(X[:, :, T - 1]).astype(np.float32)
    xrows = np.zeros((NCORES, NPCP, IN), np.float32)
    for c in range(NCORES):
        pc = perm[c * NPCP : (c + 1) * NPCP]
        m = pc >= 0
        xrows[c, m] = Xf[pc[m]]

    return dict(CW=CW, woff=woff, slots=slots, idx_tiles=idx_tiles,
                pm=pm, perm=perm, xrows=xrows)


def _build_weights(inp):
    f32 = np.float32
    W_in = np.asarray(inp["W_in"], f32)
    W1 = np.asarray(inp["W1"], f32)
    W2 = np.asarray(inp["W2"], f32)

    def att_mat(a_s, a_d):
        A = np.zeros((C, 2 * H), f32)
        for k in range(H):
            A[64 * k : 64 * (k + 1), k] = a_s[k]
            A[64 * k : 64 * (k + 1), H + k] = a_d[k]
        return A

    WA1 = W1 @ att_mat(np.asarray(inp["as1"], f32), np.asarray(inp["ad1"], f32))
    WA2 = W2 @ att_mat(np.asarray(inp["as2"], f32), np.asarray(inp["ad2"], f32))
    b_in = np.asarray(inp["b_in"], f32)
    return dict(
        W_in=W_in,
        W1s=np.ascontiguousarray(np.stack([W1[:128], W1[128:]], axis=1)),
        W2s=np.ascontiguousarray(np.stack([W2[:128], W2[128:]], axis=1)),
        WA1s=np.ascontiguousarray(np.stack([WA1[:128], WA1[128:]], axis=1)),
        WA2s=np.ascontiguousarray(np.stack([WA2[:128], WA2[128:]], axis=1)),
        b_in_cols=np.ascontiguousarray(np.stack([b_in[:128], b_in[128:]], 1)),
        g1=np.asarray(inp["g1"], f32)[None, :],
        be1=np.asarray(inp["be1"], f32)[None, :],
        g2=np.asarray(inp["g2"], f32)[None, :],
        be2=np.asarray(inp["be2"], f32)[None, :],
        Wo1=np.asarray(inp["Wo1"], f32),
        bo1=np.asarray(inp["bo1"], f32)[None, :],
        Wo2rep=np.ascontiguousarray(
            np.broadcast_to(np.asarray(inp["Wo2"], f32)[:, 0][None, :], (P, C))),
        bo2rep=np.full((P, 1), float(np.asarray(inp["bo2"]).reshape(-1)[0]), f32),
        ident=np.eye(P, dtype=f32),
        ones=np.ones((P, 1), f32),
        ones_row=np.ones((1, P), f32),
    )


def _build_program(CW, woff, slots, repeat=1, local_cc=None):
    import concourse.bacc as bacc
    import concourse.tile as tile
    from concourse import mybir

    if local_cc is None:
        local_cc = repeat > 1
    nc = bacc.Bacc("TRN2", num_devices=NCORES)
    dt = mybir.dt
    f32 = dt.float32
    AX = mybir.AxisListType
    OP = mybir.AluOpType
    ACT = mybir.ActivationFunctionType
    CCG = [list(range(NCORES))]

    d_x = nc.declare_dram_parameter("xrows", [NPCP, IN], f32, isOutput=False)
    d_idx = nc.declare_dram_parameter("idx_tiles", [P, slots // 16], dt.int16,
                                      isOutput=False)
    d_pm = nc.declare_dram_parameter("pm", [P, slots // P, 2], f32, isOutput=False)
    d_Win = nc.declare_dram_parameter("W_in", [IN, C], f32, isOutput=False)
    d_W1s = nc.declare_dram_parameter("W1s", [P, 2, C], f32, isOutput=False)
    d_W2s = nc.declare_dram_parameter("W2s", [P, 2, C], f32, isOutput=False)
    d_WA1s = nc.declare_dram_parameter("WA1s", [P, 2, 2 * H], f32, isOutput=False)
    d_WA2s = nc.declare_dram_parameter("WA2s", [P, 2, 2 * H], f32, isOutput=False)
    d_binc = nc.declare_dram_parameter("b_in_cols", [P, 2], f32, isOutput=False)
    d_g1 = nc.declare_dram_parameter("g1", [1, C], f32, isOutput=False)
    d_be1 = nc.declare_dram_parameter("be1", [1, C], f32, isOutput=False)
    d_g2 = nc.declare_dram_parameter("g2", [1, F], f32, isOutput=False)
    d_be2 = nc.declare_dram_parameter("be2", [1, F], f32, isOutput=False)
    d_Wo1 = nc.declare_dram_parameter("Wo1", [F, END], f32, isOutput=False)
    d_bo1 = nc.declare_dram_parameter("bo1", [1, END], f32, isOutput=False)
    d_Wo2r = nc.declare_dram_parameter("Wo2rep", [P, C], f32, isOutput=False)
    d_bo2r = nc.declare_dram_parameter("bo2rep", [P, 1], f32, isOutput=False)
    d_id = nc.declare_dram_parameter("ident", [P, P], f32, isOutput=False)
    d_ones = nc.declare_dram_parameter("ones", [P, 1], f32, isOutput=False)
    d_onesr = nc.declare_dram_parameter("ones_row", [1, P], f32, isOutput=False)
    d_out = nc.declare_dram_parameter("out", [NPCP, 1], f32, isOutput=True)

    loc1 = nc.dram_tensor("loc1", [NPCP, DW], f32)
    tab1 = nc.dram_tensor("tab1", [ROWS, DW], f32, addr_space="Shared")
    g1loc = nc.dram_tensor("g1loc", [NPCP, C], f32)
    loc2 = nc.dram_tensor("loc2", [NPCP, DW], f32)
    tab2 = nc.dram_tensor("tab2", [ROWS, DW], f32, addr_space="Shared")
    g2loc = nc.dram_tensor("g2loc", [NPCP, F], f32)
    st1 = nc.dram_tensor("st1", [2, C], f32)
    st1r = nc.dram_tensor("st1r", [2, C], f32, addr_space="Shared")
    st2 = nc.dram_tensor("st2", [2, F], f32)
    st2r = nc.dram_tensor("st2r", [2, F], f32, addr_space="Shared")
    sc1 = nc.dram_tensor("sc1", [2, C], f32)
    sc2 = nc.dram_tensor("sc2", [2, F], f32)

    def mm(out, lhsT, rhs, start, stop):
        nc.tensor.matmul(out=out, lhsT=lhsT, rhs=rhs, start=start, stop=stop)

    import contextlib
    with tile.TileContext(nc) as tc:
        with (
            tc.tile_pool(name="const", bufs=1) as cpool,
            tc.tile_pool(name="sbuf", bufs=2) as sbuf,
            tc.tile_pool(name="gat", bufs=3) as gpool,
            tc.tile_pool(name="msgp", bufs=2) as mpool,
            tc.tile_pool(name="psum", bufs=2, space="PSUM") as psum,
            tc.tile_pool(name="pstat", bufs=1, space="PSUM") as pstat,
        ):
            def ctile(dram, shape, tag, dtt=f32):
                t = cpool.tile(shape, dtt, tag=tag)
                nc.sync.dma_start(out=t[:], in_=dram[:])
                return t

            ident = ctile(d_id, [P, P], "ident")
            ones = ctile(d_ones, [P, 1], "ones")
            ones_r2 = cpool.tile([P, P], f32, tag="ones_r")
            nc.sync.dma_start(out=ones_r2[0:1, :], in_=d_onesr[:])
            Win_t = ctile(d_Win, [IN, C], "Win")
            W1_t = ctile(d_W1s, [P, 2, C], "W1")
            W2_t = ctile(d_W2s, [P, 2, C], "W2")
            WA1_t = ctile(d_WA1s, [P, 2, 2 * H], "WA1")
            WA2_t = ctile(d_WA2s, [P, 2, 2 * H], "WA2")
            binc_t = ctile(d_binc, [P, 2], "binc")
            Wo1_t = cpool.tile([P, END], f32, tag="Wo1")
            nc.sync.dma_start(out=Wo1_t[0:F, :], in_=d_Wo1[:])
            bo1_t = cpool.tile([P, END], f32, tag="bo1")
            nc.sync.dma_start(out=bo1_t[0:1, :], in_=d_bo1[:])
            Wo2r_t = ctile(d_Wo2r, [P, C], "Wo2r")
            bo2r_t = ctile(d_bo2r, [P, 1], "bo2r")
            idx_t = ctile(d_idx, [P, slots // 16], "idxt", dt.int16)
            pm_t = ctile(d_pm, [P, slots // P, 2], "pmt")

            rep_cm = tc.For_i(0, repeat, 1) if repeat > 1 else contextlib.nullcontext()
            with rep_cm:
                # ---------------- table-row builder -------------------------
                def build_table(rows_getter, W_t, WA_t, loc):
                    for t in range(NW):
                        yT = rows_getter(t)
                        ph = psum.tile([P, C + H], f32, space="PSUM", tag="big")
                        pa = psum.tile([P, 2 * H], f32, space="PSUM", tag="small")
                        for hf in range(2):
                            mm(ph[:, 0:C], yT[hf][:], W_t[:, hf, :],
                               start=(hf == 0), stop=(hf == 1))
                            mm(pa[:], yT[hf][:], WA_t[:, hf, :],
                               start=(hf == 0), stop=(hf == 1))
                        stg = sbuf.tile([P, DW], f32, tag="stgA")
                        nc.vector.tensor_copy(out=stg[:, 0:C], in_=ph[:, 0:C])
                        nc.vector.tensor_copy(out=stg[:, C : C + 2 * H], in_=pa[:])
                        nc.vector.memset(stg[:, C + 2 * H : DW], 0.0)
                        nc.sync.dma_start(out=loc[t * P : (t + 1) * P, :], in_=stg[:])

                # ---------------- phase A ------------------------------------
                def phaseA_rows(t):
                    xs = sbuf.tile([P, IN], f32, tag="xs")
                    nc.sync.dma_start(out=xs[:], in_=d_x[t * P : (t + 1) * P, :])
                    pt = psum.tile([P, P], f32, space="PSUM", tag="tr")
                    nc.tensor.transpose(out=pt[:], in_=xs[:], identity=ident[:])
                    xsT = sbuf.tile([P, P], f32, tag="xsT")
                    nc.vector.tensor_copy(out=xsT[:], in_=pt[:])
                    yT = []
                    for hf in range(2):
                        px = psum.tile([P, P], f32, space="PSUM", tag="tr")
                        mm(px[:], Win_t[:, hf * P : (hf + 1) * P], xsT[:],
                           start=True, stop=True)
                        xt = sbuf.tile([P, P], f32, tag=f"x0T{hf}")
                        nc.vector.tensor_tensor(
                            out=xt[:], in0=px[:],
                            in1=binc_t[:, hf : hf + 1].broadcast_to([P, P]),
                            op=OP.add)
                        yT.append(xt)
                    return yT

                build_table(phaseA_rows, W1_t, WA1_t, loc1)
                if local_cc:
                    nc.sync.dma_start(out=tab1[0:NPCP, :], in_=loc1[:])
                else:
                    nc.gpsimd.collective_compute(
                        "AllGather", OP.bypass, replica_groups=CCG,
                        ins=[loc1[:].opt()], outs=[tab1[:].opt()])

                # ---------------- edge phase ---------------------------------
                def edge_phase(tab, loc, layer):
                    outw = C if layer == 1 else F
                    pstats = pstat.tile([P, C], f32, space="PSUM", tag="sx")
                    pstats2 = pstat.tile([P, C], f32, space="PSUM", tag="sxx")
                    tabv = tab[:].rearrange("(q two) d -> q (two d)", two=2)
                    for w in range(NW):
                        cw = CW[w]
                        off = int(woff[w])
                        attD = sbuf.tile([P, H], f32, tag="attD")
                        nc.sync.dma_start(
                            out=attD[:],
                            in_=loc[w * P : (w + 1) * P, C + H : C + 2 * H])
                        po = psum.tile([P, C + H], f32, space="PSUM", tag="big")
                        nsub = (cw + CAP - 1) // CAP
                        szs = [cw // nsub + (1 if i < cw % nsub else 0)
                               for i in range(nsub)]
                        offs = [sum(szs[:i]) for i in range(nsub)]
                        for s in range(nsub):
                            c0 = offs[s]
                            ns = szs[s]
                            hg = gpool.tile([P, CAP, 2 * DW], f32, tag="hg")
                            nc.gpsimd.dma_gather(
                                out_ap=hg[:, 0:ns, :],
                                in_ap=tabv,
                                idxs_ap=idx_t[:, (off + c0) * 8 : (off + c0 + ns) * 8],
                                num_idxs=ns * P,
                                num_idxs_reg=ns * P,
                                elem_size=2 * DW,
                                single_packet=False,
                            )
                            hgv = hg[:, 0:ns, :].rearrange(
                                "p c (two d) -> p c two d", two=2)
                            ex = mpool.tile([P, CAP, 2, H], f32, tag="ex")
                            nc.vector.tensor_tensor(
                                out=ex[:, 0:ns],
                                in0=hgv[:, :, :, C : C + H],
                                in1=attD[:].unsqueeze(1).unsqueeze(1)
                                    .broadcast_to([P, ns, 2, H]),
                                op=OP.add)
                            lr = mpool.tile([P, CAP, 2, H], f32, tag="lr")
                            nc.vector.tensor_scalar(
                                out=lr[:, 0:ns], in0=ex[:, 0:ns], scalar1=0.2,
                                scalar2=None, op0=OP.mult)
                            nc.vector.tensor_tensor(
                                out=lr[:, 0:ns], in0=lr[:, 0:ns], in1=ex[:, 0:ns],
                                op=OP.max)
                            nc.scalar.activation(out=ex[:, 0:ns], in_=lr[:, 0:ns],
                                                 func=ACT.Exp)
                            nc.vector.tensor_tensor(
                                out=ex[:, 0:ns], in0=ex[:, 0:ns],
                                in1=pm_t[:, off + c0 : off + c0 + ns, :]
                                    .unsqueeze(3).broadcast_to([P, ns, 2, H]),
                                op=OP.mult)
                            for par in range(2):
                                nc.vector.tensor_tensor(
                                    out=hgv[:, :, par, 0:C].rearrange(
                                        "p c (k f) -> p c k f", k=H),
                                    in0=hgv[:, :, par, 0:C].rearrange(
                                        "p c (k f) -> p c k f", k=H),
                                    in1=ex[:, 0:ns, par, :].unsqueeze(3)
                                        .broadcast_to([P, ns, H, F]),
                                    op=OP.mult)
                            nc.vector.tensor_copy(
                                out=hgv[:, :, :, C : C + H], in_=ex[:, 0:ns])
                            if COMBINE:
                                msgc = mpool.tile([P, CAP, C + H], f32, tag="msgc")
                                nc.vector.tensor_tensor(
                                    out=msgc[:, 0:ns],
                                    in0=hgv[:, :, 0, 0 : C + H],
                                    in1=hgv[:, :, 1, 0 : C + H], op=OP.add)
                                for cc in range(ns):
                                    mm(po[:], ident[:], msgc[:, cc, :],
                                       start=(s == 0 and cc == 0),
                                       stop=(s == nsub - 1 and cc == ns - 1))
                            else:
                                for cc in range(ns):
                                    for par in range(2):
                                        mm(po[:], ident[:],
                                           hgv[:, cc, par, 0 : C + H],
                                           start=(s == 0 and cc == 0 and par == 0),
                                           stop=(s == nsub - 1 and cc == ns - 1
                                                 and par == 1))
                        # flush
                        sden = sbuf.tile([P, H], f32, tag="sden")
                        nc.vector.tensor_scalar(out=sden[:], in0=po[:, C : C + H],
                                                scalar1=1e-16, scalar2=None,
                                                op0=OP.add)
                        rs = sbuf.tile([P, H], f32, tag="rs")
                        nc.vector.reciprocal(out=rs[:], in_=sden[:])
                        if layer == 1:
                            org = sbuf.tile([P, C], f32, tag="org")
                            nc.vector.tensor_tensor(
                                out=org[:].rearrange("p (k f) -> p k f", k=H),
                                in0=po[:, 0:C].rearrange("p (k f) -> p k f", k=H),
                                in1=rs[:].unsqueeze(2).broadcast_to([P, H, F]),
                                op=OP.mult)
                            nc.sync.dma_start(out=g1loc[w * P : (w + 1) * P, :],
                                              in_=org[:])
                        else:
                            nc.vector.tensor_scalar(out=rs[:], in0=rs[:],
                                                    scalar1=0.25, scalar2=None,
                                                    op0=OP.mult)
                            tmp = sbuf.tile([P, C], f32, tag="tmp2")
                            nc.vector.tensor_tensor(
                                out=tmp[:].rearrange("p (k f) -> p k f", k=H),
                                in0=po[:, 0:C].rearrange("p (k f) -> p k f", k=H),
                                in1=rs[:].unsqueeze(2).broadcast_to([P, H, F]),
                                op=OP.mult)
                            org = sbuf.tile([P, F], f32, tag="orgf")
                            nc.vector.tensor_tensor(out=org[:], in0=tmp[:, 0:F],
                                                    in1=tmp[:, F : 2 * F], op=OP.add)
                            nc.vector.tensor_tensor(out=org[:], in0=org[:],
                                                    in1=tmp[:, 2 * F : 3 * F],
                                                    op=OP.add)
                            nc.vector.tensor_tensor(out=org[:], in0=org[:],
                                                    in1=tmp[:, 3 * F : 4 * F],
                                                    op=OP.add)
                            nc.sync.dma_start(out=g2loc[w * P : (w + 1) * P, :],
                                              in_=org[:])
                    # ---- stats readback pass (off the edge-phase critical
                    # path: avoids stalling PE on each window flush) ----
                    gsrc = g1loc if layer == 1 else g2loc
                    for t in range(NW):
                        gr = sbuf.tile([P, C], f32, tag="gstat")
                        nc.sync.dma_start(out=gr[:, 0:outw],
                                          in_=gsrc[t * P : (t + 1) * P, :])
                        sq = sbuf.tile([P, C], f32, tag="sq")
                        nc.vector.tensor_tensor(out=sq[:, 0:outw],
                                                in0=gr[:, 0:outw],
                                                in1=gr[:, 0:outw], op=OP.mult)
                        mm(pstats[0:1, 0:outw], ones[:], gr[:, 0:outw],
                           start=(t == 0), stop=(t == NW - 1))
                        mm(pstats2[0:1, 0:outw], ones[:], sq[:, 0:outw],
                           start=(t == 0), stop=(t == NW - 1))
                    # moments -> AllReduce -> scale/shift rows in DRAM
                    stg0 = sbuf.tile([P, C], f32, tag="stg0")
                    nc.vector.tensor_copy(out=stg0[0:1, 0:outw],
                                          in_=pstats[0:1, 0:outw])
                    stg1 = sbuf.tile([P, C], f32, tag="stg1")
                    nc.vector.tensor_copy(out=stg1[0:1, 0:outw],
                                          in_=pstats2[0:1, 0:outw])
                    std = st1 if layer == 1 else st2
                    stdr = st1r if layer == 1 else st2r
                    nc.sync.dma_start(out=std[0:1, :], in_=stg0[0:1, 0:outw])
                    nc.sync.dma_start(out=std[1:2, :], in_=stg1[0:1, 0:outw])
                    if local_cc:
                        nc.sync.dma_start(out=stdr[:, :], in_=std[:])
                    else:
                        nc.gpsimd.collective_compute(
                            "AllReduce", OP.add, replica_groups=CCG,
                            ins=[std[:].opt()], outs=[stdr[:].opt()])
                    # single-partition workspace: slices share one partition
                    bn = cpool.tile([1, 10 * C], f32, tag="bn")
                    r0 = bn[:, 0 * C : 0 * C + outw]
                    r1 = bn[:, 1 * C : 1 * C + outw]
                    gv = bn[:, 2 * C : 2 * C + outw]
                    bev = bn[:, 3 * C : 3 * C + outw]
                    mu = bn[:, 4 * C : 4 * C + outw]
                    var = bn[:, 5 * C : 5 * C + outw]
                    msq = bn[:, 6 * C : 6 * C + outw]
                    rstd = bn[:, 7 * C : 7 * C + outw]
                    scl = bn[:, 8 * C : 8 * C + outw]
                    shf = bn[:, 9 * C : 9 * C + outw]
                    nc.sync.dma_start(out=r0, in_=stdr[0:1, :])
                    nc.sync.dma_start(out=r1, in_=stdr[1:2, :])
                    nc.sync.dma_start(out=gv, in_=(d_g1 if layer == 1 else d_g2)[:])
                    nc.sync.dma_start(out=bev, in_=(d_be1 if layer == 1 else d_be2)[:])
                    nc.vector.tensor_scalar(out=mu, in0=r0, scalar1=1.0 / N,
                                            scalar2=None, op0=OP.mult)
                    nc.vector.tensor_scalar(out=var, in0=r1, scalar1=1.0 / N,
                                            scalar2=None, op0=OP.mult)
                    nc.vector.tensor_tensor(out=msq, in0=mu, in1=mu, op=OP.mult)
                    nc.vector.tensor_tensor(out=var, in0=var, in1=msq, op=OP.subtract)
                    nc.vector.tensor_scalar(out=var, in0=var, scalar1=EPS,
                                            scalar2=None, op0=OP.add)
                    nc.scalar.activation(out=msq, in_=var, func=ACT.Sqrt)
                    nc.vector.reciprocal(out=rstd, in_=msq)
                    nc.vector.tensor_tensor(out=scl, in0=gv, in1=rstd, op=OP.mult)
                    nc.vector.tensor_tensor(out=shf, in0=mu, in1=scl, op=OP.mult)
                    nc.vector.tensor_tensor(out=shf, in0=bev, in1=shf, op=OP.subtract)
                    scd = sc1 if layer == 1 else sc2
                    nc.sync.dma_start(out=scd[0:1, :], in_=scl)
                    nc.sync.dma_start(out=scd[1:2, :], in_=shf)

                edge_phase(tab1, loc1, 1)

                # ---------------- phase E ------------------------------------
                sccol1 = sbuf.tile([P, 4], f32, tag="sccol1")
                nc.sync.dma_start(
                    out=sccol1[:].rearrange("p (r h) -> p r h", r=2),
                    in_=sc1[:].rearrange("r (h p) -> p r h", p=P))

                def phaseE_rows(t):
                    g1r = sbuf.tile([P, C], f32, tag="g1r")
                    nc.sync.dma_start(out=g1r[:], in_=g1loc[t * P : (t + 1) * P, :])
                    yT = []
                    for hf in range(2):
                        ptt = psum.tile([P, P], f32, space="PSUM", tag="tr")
                        nc.tensor.transpose(out=ptt[:],
                                            in_=g1r[:, hf * P : (hf + 1) * P],
                                            identity=ident[:])
                        yt = sbuf.tile([P, P], f32, tag=f"yT{hf}")
                        nc.vector.tensor_scalar(
                            out=yt[:], in0=ptt[:],
                            scalar1=sccol1[:, hf : hf + 1],
                            scalar2=sccol1[:, 2 + hf : 3 + hf],
                            op0=OP.mult, op1=OP.add)
                        nc.vector.tensor_scalar(out=yt[:], in0=yt[:], scalar1=0.0,
                                                scalar2=None, op0=OP.max)
                        yT.append(yt)
                    return yT

                build_table(phaseE_rows, W2_t, WA2_t, loc2)
                if local_cc:
                    nc.sync.dma_start(out=tab2[0:NPCP, :], in_=loc2[:])
                else:
                    nc.gpsimd.collective_compute(
                        "AllGather", OP.bypass, replica_groups=CCG,
                        ins=[loc2[:].opt()], outs=[tab2[:].opt()])

                edge_phase(tab2, loc2, 2)

                # ---------------- phase I ------------------------------------
                sccol2 = sbuf.tile([P, 2], f32, tag="sccol2")
                nc.sync.dma_start(out=sccol2[0:F, :],
                                  in_=sc2[:].rearrange("r f -> f r"))
                for t in range(NW):
                    g2r = sbuf.tile([P, F], f32, tag="g2r")
                    nc.sync.dma_start(out=g2r[:], in_=g2loc[t * P : (t + 1) * P, :])
                    ptt = psum.tile([P, P], f32, space="PSUM", tag="tr")
                    nc.tensor.transpose(out=ptt[0:F, :], in_=g2r[:],
                                        identity=ident[:])
                    y2T = sbuf.tile([P, P], f32, tag="y2T")
                    nc.vector.tensor_scalar(
                        out=y2T[0:F, :], in0=ptt[0:F, :],
                        scalar1=sccol2[0:F, 0:1], scalar2=sccol2[0:F, 1:2],
                        op0=OP.mult, op1=OP.add)
                    pz = psum.tile([P, END], f32, space="PSUM", tag="big")
                    mm(pz[:], y2T[0:F, :], Wo1_t[0:F, :], start=True, stop=False)
                    mm(pz[:], ones_r2[0:1, :], bo1_t[0:1, :], start=False, stop=True)
                    zr = sbuf.tile([P, END], f32, tag="zr")
                    nc.vector.tensor_scalar(out=zr[:], in0=pz[:], scalar1=0.0,
                                            scalar2=None, op0=OP.max)
                    zw = sbuf.tile([P, C], f32, tag="zw")
                    nc.vector.tensor_tensor(out=zw[:], in0=zr[:], in1=Wo2r_t[:],
                                            op=OP.mult)
                    res = sbuf.tile([P, 1], f32, tag="res")
                    nc.vector.tensor_reduce(out=res[:], in_=zw[:], axis=AX.X,
                                            op=OP.add)
                    nc.vector.tensor_tensor(out=res[:], in0=res[:], in1=bo2r_t[:],
                                            op=OP.add)
                    nc.sync.dma_start(out=d_out[t * P : (t + 1) * P, :], in_=res[:])

    nc.compile()
    return nc


def kernel(**inputs):
    X = np.asarray(inputs["X"], np.float32)
    prep = _host_prep(X, inputs["edge_index"])
    wts = _build_weights(inputs)

    key = ("prog", tuple(prep["CW"]))
    if key not in _CACHE:
        _CACHE.clear()
        _CACHE[key] = _build_program(prep["CW"], prep["woff"], prep["slots"])
    nc = _CACHE[key]

    in_maps = []
    for c in range(NCORES):
        m = dict(
            xrows=prep["xrows"][c],
            idx_tiles=prep["idx_tiles"][c],
            pm=prep["pm"][c],
        )
        m.update(wts)
        in_maps.append(m)

    from concourse.bass_utils import run_bass_kernel_spmd
    res = run_bass_kernel_spmd(nc, in_maps, list(range(NCORES)))

    out = np.zeros((N, 1), np.float32)
    for c in range(NCORES):
        pc = prep["perm"][c * NPCP : (c + 1) * NPCP]
        m = pc >= 0
        out[pc[m]] = res.results[c]["out"][m, :]
    return out



# revision 11
# speedup vs baseline: 1.3753x; 1.0655x over previous
"""GAT (2-layer, 4-head) message-passing kernel for 8 Trainium2 NeuronCores.

Sharding: nodes split into 8 contiguous ranges of 6250 (padded to 6272); within
each core nodes are sorted by in-degree into 49 windows of 128 (one dst node
per SBUF partition). Each core builds hidden-table rows (h | a_s | a_d) for its
nodes, the table is AllGathered, and each core processes its own in-edges:
edge slot (p, c) = c-th in-edge of the window's p-th node. h[src] rows are
fetched with dma_gather using int16 PAIR row indices (2x320 f32 = 2560B
descriptors); a parity mask zeroes the unused pair half. Per-edge softmax
weights ex = exp(leakyrelu(a_s[src]+a_d[dst])) multiply the messages on DVE,
and identity-weight matmuls accumulate the per-partition sums in PSUM (with ex
riding along as 4 extra columns -> softmax denominators). Normalization, head
mean, batchnorm moments (ones-matmuls + 2xC AllReduce) and the MLP head follow.
Biases b1/b2 cancel inside the following batchnorms and are dropped.
"""

import numpy as np

N = 50000
E = 800000
IN = 128
T = 8
H = 4
F = 64
C = 256
END = 256
NCORES = 8
NPC = 6250
NPCP = 6272
NW = NPCP // 128
P = 128
DW = 288              # table row: 256 h | 4 a_s | 4 a_d | 24 pad
ROWS = NCORES * NPCP
EPS = 1e-5
CAP = 10              # max slot-columns per gather chunk
COMBINE = True        # pre-combine parity slabs on Pool before the PE matmul

_CACHE = {}


def _host_prep(X, edge_index):
    ei = np.asarray(edge_index)
    src = ei[0].astype(np.int64)
    dst = ei[1].astype(np.int64)
    deg = np.bincount(dst, minlength=N)

    # global degree-desc ranking: rank r -> window r//1024, core (r%1024)//128,
    # lane r%128.  All cores share each window's degree profile, so the shared
    # per-window max (CW) tracks the true degrees tightly.
    order = np.argsort(-deg, kind="stable")
    perm = np.empty(NCORES * NPCP, np.int64)
    perm.fill(-1)
    tpos = np.empty(N, np.int64)
    r = np.arange(N)
    w_of = r // (NCORES * P)
    core_of = (r % (NCORES * P)) // P
    lane_of = r % P
    pos = core_of * NPCP + w_of * P + lane_of
    perm[pos] = order
    tpos[order] = pos

    stp = tpos[src]
    dtp = tpos[dst]
    dcore = dtp // NPCP
    dlocal = dtp % NPCP

    degs = np.zeros(NCORES * NPCP, np.int64)
    degs[tpos[np.arange(N)]] = deg
    cw = degs.reshape(NCORES, NW, P).max(axis=2)
    CW = [int(x) for x in np.maximum(cw.max(axis=0), 1)]
    woff = np.concatenate([[0], np.cumsum(np.array(CW, np.int64))])
    slots = int(woff[-1]) * P

    order = np.lexsort((stp, dtp))
    sdtp, sstp = dtp[order], stp[order]
    sdcore, sdlocal = dcore[order], dlocal[order]
    uniq, counts = np.unique(sdtp, return_counts=True)
    ranks = np.arange(E) - np.repeat(np.cumsum(counts) - counts, counts)

    w = sdlocal // P
    p = sdlocal % P
    slot = (woff[w] + ranks) * P + p

    idx_pair = np.zeros((NCORES, slots), np.int16)
    pmask = np.zeros((NCORES, slots, 2), np.float32)
    for c in range(NCORES):
        m = sdcore == c
        sl = slot[m]
        st = sstp[m]
        idx_pair[c, sl] = (st // 2).astype(np.int16)
        pmask[c, sl, 0] = (st % 2 == 0).astype(np.float32)
        pmask[c, sl, 1] = (st % 2 == 1).astype(np.float32)

    def pack16(a):
        b = a.reshape(-1, 16).T
        return np.tile(b, (8, 1))

    idx_tiles = np.stack([pack16(idx_pair[c]) for c in range(NCORES)])
    pm = pmask.reshape(NCORES, slots // P, P, 2).transpose(0, 2, 1, 3).copy()

    Xf = np.ascontiguousarray(X[:, :, T - 1]).astype(np.float32)
    xrows = np.zeros((NCORES, NPCP, IN), np.float32)
    for c in range(NCORES):
        pc = perm[c * NPCP : (c + 1) * NPCP]
        m = pc >= 0
        xrows[c, m] = Xf[pc[m]]

    return dict(CW=CW, woff=woff, slots=slots, idx_tiles=idx_tiles,
                pm=pm, perm=perm, xrows=xrows)


def _build_weights(inp):
    f32 = np.float32
    W_in = np.asarray(inp["W_in"], f32)
    W1 = np.asarray(inp["W1"], f32)
    W2 = np.asarray(inp["W2"], f32)

    def att_mat(a_s, a_d):
        A = np.zeros((C, 2 * H), f32)
        for k in range(H):
            A[64 * k : 64 * (k + 1), k] = a_s[k]
            A[64 * k : 64 * (k + 1), H + k] = a_d[k]
        return A

    WA1 = W1 @ att_mat(np.asarray(inp["as1"], f32), np.asarray(inp["ad1"], f32))
    WA2 = W2 @ att_mat(np.asarray(inp["as2"], f32), np.asarray(inp["ad2"], f32))
    b_in = np.asarray(inp["b_in"], f32)
    return dict(
        W_in=W_in,
        W1s=np.ascontiguousarray(np.stack([W1[:128], W1[128:]], axis=1)),
        W2s=np.ascontiguousarray(np.stack([W2[:128], W2[128:]], axis=1)),
        WA1s=np.ascontiguousarray(np.stack([WA1[:128], WA1[128:]], axis=1)),
        WA2s=np.ascontiguousarray(np.stack([WA2[:128], WA2[128:]], axis=1)),
        b_in_cols=np.ascontiguousarray(np.stack([b_in[:128], b_in[128:]], 1)),
        g1=np.asarray(inp["g1"], f32)[None, :],
        be1=np.asarray(inp["be1"], f32)[None, :],
        g2=np.asarray(inp["g2"], f32)[None, :],
        be2=np.asarray(inp["be2"], f32)[None, :],
        Wo1=np.asarray(inp["Wo1"], f32),
        bo1=np.asarray(inp["bo1"], f32)[None, :],
        Wo2rep=np.ascontiguousarray(
            np.broadcast_to(np.asarray(inp["Wo2"], f32)[:, 0][None, :], (P, C))),
        bo2rep=np.full((P, 1), float(np.asarray(inp["bo2"]).reshape(-1)[0]), f32),
        ident=np.eye(P, dtype=f32),
        ones=np.ones((P, 1), f32),
        ones_row=np.ones((1, P), f32),
    )


def _build_program(CW, woff, slots, repeat=1, local_cc=None):
    import concourse.bacc as bacc
    import concourse.tile as tile
    from concourse import mybir

    if local_cc is None:
        local_cc = repeat > 1
    nc = bacc.Bacc("TRN2", num_devices=NCORES)
    dt = mybir.dt
    f32 = dt.float32
    AX = mybir.AxisListType
    OP = mybir.AluOpType
    ACT = mybir.ActivationFunctionType
    CCG = [list(range(NCORES))]

    d_x = nc.declare_dram_parameter("xrows", [NPCP, IN], f32, isOutput=False)
    d_idx = nc.declare_dram_parameter("idx_tiles", [P, slots // 16], dt.int16,
                                      isOutput=False)
    d_pm = nc.declare_dram_parameter("pm", [P, slots // P, 2], f32, isOutput=False)
    d_Win = nc.declare_dram_parameter("W_in", [IN, C], f32, isOutput=False)
    d_W1s = nc.declare_dram_parameter("W1s", [P, 2, C], f32, isOutput=False)
    d_W2s = nc.declare_dram_parameter("W2s", [P, 2, C], f32, isOutput=False)
    d_WA1s = nc.declare_dram_parameter("WA1s", [P, 2, 2 * H], f32, isOutput=False)
    d_WA2s = nc.declare_dram_parameter("WA2s", [P, 2, 2 * H], f32, isOutput=False)
    d_binc = nc.declare_dram_parameter("b_in_cols", [P, 2], f32, isOutput=False)
    d_g1 = nc.declare_dram_parameter("g1", [1, C], f32, isOutput=False)
    d_be1 = nc.declare_dram_parameter("be1", [1, C], f32, isOutput=False)
    d_g2 = nc.declare_dram_parameter("g2", [1, F], f32, isOutput=False)
    d_be2 = nc.declare_dram_parameter("be2", [1, F], f32, isOutput=False)
    d_Wo1 = nc.declare_dram_parameter("Wo1", [F, END], f32, isOutput=False)
    d_bo1 = nc.declare_dram_parameter("bo1", [1, END], f32, isOutput=False)
    d_Wo2r = nc.declare_dram_parameter("Wo2rep", [P, C], f32, isOutput=False)
    d_bo2r = nc.declare_dram_parameter("bo2rep", [P, 1], f32, isOutput=False)
    d_id = nc.declare_dram_parameter("ident", [P, P], f32, isOutput=False)
    d_ones = nc.declare_dram_parameter("ones", [P, 1], f32, isOutput=False)
    d_onesr = nc.declare_dram_parameter("ones_row", [1, P], f32, isOutput=False)
    d_out = nc.declare_dram_parameter("out", [NPCP, 1], f32, isOutput=True)

    loc1 = nc.dram_tensor("loc1", [NPCP, DW], f32)
    tab1 = nc.dram_tensor("tab1", [ROWS, DW], f32, addr_space="Shared")
    g1loc = nc.dram_tensor("g1loc", [NPCP, C], f32)
    loc2 = nc.dram_tensor("loc2", [NPCP, DW], f32)
    tab2 = nc.dram_tensor("tab2", [ROWS, DW], f32, addr_space="Shared")
    g2loc = nc.dram_tensor("g2loc", [NPCP, F], f32)
    st1 = nc.dram_tensor("st1", [2, C], f32)
    st1r = nc.dram_tensor("st1r", [2, C], f32, addr_space="Shared")
    st2 = nc.dram_tensor("st2", [2, F], f32)
    st2r = nc.dram_tensor("st2r", [2, F], f32, addr_space="Shared")
    sc1 = nc.dram_tensor("sc1", [2, C], f32)
    sc2 = nc.dram_tensor("sc2", [2, F], f32)

    def mm(out, lhsT, rhs, start, stop):
        nc.tensor.matmul(out=out, lhsT=lhsT, rhs=rhs, start=start, stop=stop)

    import contextlib
    with tile.TileContext(nc) as tc:
        with (
            tc.tile_pool(name="const", bufs=1) as cpool,
            tc.tile_pool(name="sbuf", bufs=2) as sbuf,
            tc.tile_pool(name="gat", bufs=3) as gpool,
            tc.tile_pool(name="msgp", bufs=2) as mpool,
            tc.tile_pool(name="psum", bufs=2, space="PSUM") as psum,
            tc.tile_pool(name="psumb", bufs=3, space="PSUM") as psumb,
            tc.tile_pool(name="pstat", bufs=1, space="PSUM") as pstat,
        ):
            def ctile(dram, shape, tag, dtt=f32):
                t = cpool.tile(shape, dtt, tag=tag)
                nc.sync.dma_start(out=t[:], in_=dram[:])
                return t

            ident = ctile(d_id, [P, P], "ident")
            ones = ctile(d_ones, [P, 1], "ones")
            ones_r2 = cpool.tile([P, P], f32, tag="ones_r")
            nc.sync.dma_start(out=ones_r2[0:1, :], in_=d_onesr[:])
            Win_t = ctile(d_Win, [IN, C], "Win")
            W1_t = ctile(d_W1s, [P, 2, C], "W1")
            W2_t = ctile(d_W2s, [P, 2, C], "W2")
            WA1_t = ctile(d_WA1s, [P, 2, 2 * H], "WA1")
            WA2_t = ctile(d_WA2s, [P, 2, 2 * H], "WA2")
            binc_t = ctile(d_binc, [P, 2], "binc")
            Wo1_t = cpool.tile([P, END], f32, tag="Wo1")
            nc.sync.dma_start(out=Wo1_t[0:F, :], in_=d_Wo1[:])
            bo1_t = cpool.tile([P, END], f32, tag="bo1")
            nc.sync.dma_start(out=bo1_t[0:1, :], in_=d_bo1[:])
            Wo2r_t = ctile(d_Wo2r, [P, C], "Wo2r")
            bo2r_t = ctile(d_bo2r, [P, 1], "bo2r")
            idx_t = ctile(d_idx, [P, slots // 16], "idxt", dt.int16)
            pm_t = ctile(d_pm, [P, slots // P, 2], "pmt")

            rep_cm = tc.For_i(0, repeat, 1) if repeat > 1 else contextlib.nullcontext()
            with rep_cm:
                # ---------------- table-row builder -------------------------
                def build_table(rows_getter, W_t, WA_t, loc):
                    for t in range(NW):
                        yT = rows_getter(t)
                        ph = psumb.tile([P, C + 2 * H], f32, space="PSUM", tag="big")
                        for hf in range(2):
                            mm(ph[:, 0:C], yT[hf][:], W_t[:, hf, :],
                               start=(hf == 0), stop=(hf == 1))
                            nc.tensor.matmul(out=ph[:, C : C + 2 * H],
                                             lhsT=yT[hf][:], rhs=WA_t[:, hf, :],
                                             start=False, stop=False,
                                             skip_group_check=True)
                        stg = sbuf.tile([P, DW], f32, tag="stgA")
                        nc.vector.tensor_copy(out=stg[:, 0 : C + 2 * H],
                                              in_=ph[:, 0 : C + 2 * H])
                        nc.vector.memset(stg[:, C + 2 * H : DW], 0.0)
                        nc.sync.dma_start(out=loc[t * P : (t + 1) * P, :], in_=stg[:])

                # ---------------- phase A ------------------------------------
                def phaseA_rows(t):
                    xs = sbuf.tile([P, IN], f32, tag="xs")
                    nc.sync.dma_start(out=xs[:], in_=d_x[t * P : (t + 1) * P, :])
                    pt = psum.tile([P, P], f32, space="PSUM", tag="tr")
                    nc.tensor.transpose(out=pt[:], in_=xs[:], identity=ident[:])
                    xsT = sbuf.tile([P, P], f32, tag="xsT")
                    nc.vector.tensor_copy(out=xsT[:], in_=pt[:])
                    yT = []
                    for hf in range(2):
                        px = psum.tile([P, P], f32, space="PSUM", tag="tr")
                        mm(px[:], Win_t[:, hf * P : (hf + 1) * P], xsT[:],
                           start=True, stop=True)
                        xt = sbuf.tile([P, P], f32, tag=f"x0T{hf}")
                        nc.vector.tensor_tensor(
                            out=xt[:], in0=px[:],
                            in1=binc_t[:, hf : hf + 1].broadcast_to([P, P]),
                            op=OP.add)
                        yT.append(xt)
                    return yT

                build_table(phaseA_rows, W1_t, WA1_t, loc1)
                if local_cc:
                    nc.sync.dma_start(out=tab1[0:NPCP, :], in_=loc1[:])
                else:
                    nc.gpsimd.collective_compute(
                        "AllGather", OP.bypass, replica_groups=CCG,
                        ins=[loc1[:].opt()], outs=[tab1[:].opt()])

                # ---------------- edge phase ---------------------------------
                def edge_phase(tab, loc, layer):
                    outw = C if layer == 1 else F
                    pstats = pstat.tile([P, C], f32, space="PSUM", tag="sx")
                    pstats2 = pstat.tile([P, C], f32, space="PSUM", tag="sxx")
                    tabv = tab[:].rearrange("(q two) d -> q (two d)", two=2)

                    def flush(w, po):
                        sden = sbuf.tile([P, H], f32, tag="sden")
                        nc.vector.tensor_scalar(out=sden[:], in0=po[:, C : C + H],
                                                scalar1=1e-16, scalar2=None,
                                                op0=OP.add)
                        rs = sbuf.tile([P, H], f32, tag="rs")
                        nc.vector.reciprocal(out=rs[:], in_=sden[:])
                        if layer == 1:
                            org = sbuf.tile([P, C], f32, tag="org")
                            nc.vector.tensor_tensor(
                                out=org[:].rearrange("p (k f) -> p k f", k=H),
                                in0=po[:, 0:C].rearrange("p (k f) -> p k f", k=H),
                                in1=rs[:].unsqueeze(2).broadcast_to([P, H, F]),
                                op=OP.mult)
                            nc.sync.dma_start(out=g1loc[w * P : (w + 1) * P, :],
                                              in_=org[:])
                        else:
                            nc.vector.tensor_scalar(out=rs[:], in0=rs[:],
                                                    scalar1=0.25, scalar2=None,
                                                    op0=OP.mult)
                            tmp = sbuf.tile([P, C], f32, tag="tmp2")
                            nc.vector.tensor_tensor(
                                out=tmp[:].rearrange("p (k f) -> p k f", k=H),
                                in0=po[:, 0:C].rearrange("p (k f) -> p k f", k=H),
                                in1=rs[:].unsqueeze(2).broadcast_to([P, H, F]),
                                op=OP.mult)
                            org = sbuf.tile([P, F], f32, tag="orgf")
                            nc.vector.tensor_tensor(out=org[:], in0=tmp[:, 0:F],
                                                    in1=tmp[:, F : 2 * F], op=OP.add)
                            nc.vector.tensor_tensor(out=org[:], in0=org[:],
                                                    in1=tmp[:, 2 * F : 3 * F],
                                                    op=OP.add)
                            nc.vector.tensor_tensor(out=org[:], in0=org[:],
                                                    in1=tmp[:, 3 * F : 4 * F],
                                                    op=OP.add)
                            nc.sync.dma_start(out=g2loc[w * P : (w + 1) * P, :],
                                              in_=org[:])

                    pend = []
                    for w in range(NW):
                        cw = CW[w]
                        off = int(woff[w])
                        attD = sbuf.tile([P, H], f32, tag="attD")
                        nc.sync.dma_start(
                            out=attD[:],
                            in_=loc[w * P : (w + 1) * P, C + H : C + 2 * H])
                        po = psumb.tile([P, C + 2 * H], f32, space="PSUM",
                                        tag="big")
                        nsub = (cw + CAP - 1) // CAP
                        szs = [cw // nsub + (1 if i < cw % nsub else 0)
                               for i in range(nsub)]
                        offs = [sum(szs[:i]) for i in range(nsub)]
                        for s in range(nsub):
                            c0 = offs[s]
                            ns = szs[s]
                            hg = gpool.tile([P, CAP, 2 * DW], f32, tag="hg")
                            nc.gpsimd.dma_gather(
                                out_ap=hg[:, 0:ns, :],
                                in_ap=tabv,
                                idxs_ap=idx_t[:, (off + c0) * 8 : (off + c0 + ns) * 8],
                                num_idxs=ns * P,
                                num_idxs_reg=ns * P,
                                elem_size=2 * DW,
                                single_packet=False,
                            )
                            hgv = hg[:, 0:ns, :].rearrange(
                                "p c (two d) -> p c two d", two=2)
                            ex = mpool.tile([P, CAP, 2, H], f32, tag="ex")
                            nc.vector.tensor_tensor(
                                out=ex[:, 0:ns],
                                in0=hgv[:, :, :, C : C + H],
                                in1=attD[:].unsqueeze(1).unsqueeze(1)
                                    .broadcast_to([P, ns, 2, H]),
                                op=OP.add)
                            lr = mpool.tile([P, CAP, 2, H], f32, tag="lr")
                            nc.vector.tensor_scalar(
                                out=lr[:, 0:ns], in0=ex[:, 0:ns], scalar1=0.2,
                                scalar2=None, op0=OP.mult)
                            nc.vector.tensor_tensor(
                                out=lr[:, 0:ns], in0=lr[:, 0:ns], in1=ex[:, 0:ns],
                                op=OP.max)
                            nc.scalar.activation(out=ex[:, 0:ns], in_=lr[:, 0:ns],
                                                 func=ACT.Exp)
                            nc.vector.tensor_tensor(
                                out=hgv[:, :, :, C : C + H],
                                in0=ex[:, 0:ns],
                                in1=pm_t[:, off + c0 : off + c0 + ns, :]
                                    .unsqueeze(3).broadcast_to([P, ns, 2, H]),
                                op=OP.mult)
                            for par in range(2):
                                nc.vector.tensor_tensor(
                                    out=hgv[:, :, par, 0:C].rearrange(
                                        "p c (k f) -> p c k f", k=H),
                                    in0=hgv[:, :, par, 0:C].rearrange(
                                        "p c (k f) -> p c k f", k=H),
                                    in1=hgv[:, :, par, C : C + H].unsqueeze(3)
                                        .broadcast_to([P, ns, H, F]),
                                    op=OP.mult)
                            for par in range(2):
                                for cc in range(ns):
                                    mm(po[:, 0 : C + H], ident[:],
                                       hgv[:, cc, par, 0 : C + H],
                                       start=(s == 0 and par == 0 and cc == 0),
                                       stop=(s == nsub - 1 and par == 1
                                             and cc == ns - 1))
                        pend.append((w, po))
                        if len(pend) == 2:
                            flush(*pend.pop(0))
                    for item in pend:
                        flush(*item)
                    # ---- stats readback pass (off the edge-phase critical
                    # path: avoids stalling PE on each window flush) ----
                    gsrc = g1loc if layer == 1 else g2loc
                    for t in range(NW):
                        gr = sbuf.tile([P, C], f32, tag="gstat")
                        nc.sync.dma_start(out=gr[:, 0:outw],
                                          in_=gsrc[t * P : (t + 1) * P, :])
                        sq = sbuf.tile([P, C], f32, tag="sq")
                        nc.vector.tensor_tensor(out=sq[:, 0:outw],
                                                in0=gr[:, 0:outw],
                                                in1=gr[:, 0:outw], op=OP.mult)
                        mm(pstats[0:1, 0:outw], ones[:], gr[:, 0:outw],
                           start=(t == 0), stop=(t == NW - 1))
                        mm(pstats2[0:1, 0:outw], ones[:], sq[:, 0:outw],
                           start=(t == 0), stop=(t == NW - 1))
                    # moments -> AllReduce -> scale/shift rows in DRAM
                    stg0 = sbuf.tile([P, C], f32, tag="stg0")
                    nc.vector.tensor_copy(out=stg0[0:1, 0:outw],
                                          in_=pstats[0:1, 0:outw])
                    stg1 = sbuf.tile([P, C], f32, tag="stg1")
                    nc.vector.tensor_copy(out=stg1[0:1, 0:outw],
                                          in_=pstats2[0:1, 0:outw])
                    std = st1 if layer == 1 else st2
                    stdr = st1r if layer == 1 else st2r
                    nc.sync.dma_start(out=std[0:1, :], in_=stg0[0:1, 0:outw])
                    nc.sync.dma_start(out=std[1:2, :], in_=stg1[0:1, 0:outw])
                    if local_cc:
                        nc.sync.dma_start(out=stdr[:, :], in_=std[:])
                    else:
                        nc.gpsimd.collective_compute(
                            "AllReduce", OP.add, replica_groups=CCG,
                            ins=[std[:].opt()], outs=[stdr[:].opt()])
                    # single-partition workspace: slices share one partition
                    bn = cpool.tile([1, 10 * C], f32, tag="bn")
                    r0 = bn[:, 0 * C : 0 * C + outw]
                    r1 = bn[:, 1 * C : 1 * C + outw]
                    gv = bn[:, 2 * C : 2 * C + outw]
                    bev = bn[:, 3 * C : 3 * C + outw]
                    mu = bn[:, 4 * C : 4 * C + outw]
                    var = bn[:, 5 * C : 5 * C + outw]
                    msq = bn[:, 6 * C : 6 * C + outw]
                    rstd = bn[:, 7 * C : 7 * C + outw]
                    scl = bn[:, 8 * C : 8 * C + outw]
                    shf = bn[:, 9 * C : 9 * C + outw]
                    nc.sync.dma_start(out=r0, in_=stdr[0:1, :])
                    nc.sync.dma_start(out=r1, in_=stdr[1:2, :])
                    nc.sync.dma_start(out=gv, in_=(d_g1 if layer == 1 else d_g2)[:])
                    nc.sync.dma_start(out=bev, in_=(d_be1 if layer == 1 else d_be2)[:])
                    nc.vector.tensor_scalar(out=mu, in0=r0, scalar1=1.0 / N,
                                            scalar2=None, op0=OP.mult)
                    nc.vector.tensor_scalar(out=var, in0=r1, scalar1=1.0 / N,
                                            scalar2=None, op0=OP.mult)
                    nc.vector.tensor_tensor(out=msq, in0=mu, in1=mu, op=OP.mult)
                    nc.vector.tensor_tensor(out=var, in0=var, in1=msq, op=OP.subtract)
                    nc.vector.tensor_scalar(out=var, in0=var, scalar1=EPS,
                                            scalar2=None, op0=OP.add)
                    nc.scalar.activation(out=msq, in_=var, func=ACT.Sqrt)
                    nc.vector.reciprocal(out=rstd, in_=msq)
                    nc.vector.tensor_tensor(out=scl, in0=gv, in1=rstd, op=OP.mult)
                    nc.vector.tensor_tensor(out=shf, in0=mu, in1=scl, op=OP.mult)
                    nc.vector.tensor_tensor(out=shf, in0=bev, in1=shf, op=OP.subtract)
                    scd = sc1 if layer == 1 else sc2
                    nc.sync.dma_start(out=scd[0:1, :], in_=scl)
                    nc.sync.dma_start(out=scd[1:2, :], in_=shf)

                edge_phase(tab1, loc1, 1)

                # ---------------- phase E ------------------------------------
                sccol1 = sbuf.tile([P, 4], f32, tag="sccol1")
                nc.sync.dma_start(
                    out=sccol1[:].rearrange("p (r h) -> p r h", r=2),
                    in_=sc1[:].rearrange("r (h p) -> p r h", p=P))

                def phaseE_rows(t):
                    g1r = sbuf.tile([P, C], f32, tag="g1r")
                    nc.sync.dma_start(out=g1r[:], in_=g1loc[t * P : (t + 1) * P, :])
                    yT = []
                    for hf in range(2):
                        ptt = psum.tile([P, P], f32, space="PSUM", tag="tr")
                        nc.tensor.transpose(out=ptt[:],
                                            in_=g1r[:, hf * P : (hf + 1) * P],
                                            identity=ident[:])
                        yt = sbuf.tile([P, P], f32, tag=f"yT{hf}")
                        nc.vector.tensor_scalar(
                            out=yt[:], in0=ptt[:],
                            scalar1=sccol1[:, hf : hf + 1],
                            scalar2=sccol1[:, 2 + hf : 3 + hf],
                            op0=OP.mult, op1=OP.add)
                        nc.vector.tensor_scalar(out=yt[:], in0=yt[:], scalar1=0.0,
                                                scalar2=None, op0=OP.max)
                        yT.append(yt)
                    return yT

                build_table(phaseE_rows, W2_t, WA2_t, loc2)
                if local_cc:
                    nc.sync.dma_start(out=tab2[0:NPCP, :], in_=loc2[:])
                else:
                    nc.gpsimd.collective_compute(
                        "AllGather", OP.bypass, replica_groups=CCG,
                        ins=[loc2[:].opt()], outs=[tab2[:].opt()])

                edge_phase(tab2, loc2, 2)

                # ---------------- phase I ------------------------------------
                sccol2 = sbuf.tile([P, 2], f32, tag="sccol2")
                nc.sync.dma_start(out=sccol2[0:F, :],
                                  in_=sc2[:].rearrange("r f -> f r"))
                for t in range(NW):
                    g2r = sbuf.tile([P, F], f32, tag="g2r")
                    nc.sync.dma_start(out=g2r[:], in_=g2loc[t * P : (t + 1) * P, :])
                    ptt = psum.tile([P, P], f32, space="PSUM", tag="tr")
                    nc.tensor.transpose(out=ptt[0:F, :], in_=g2r[:],
                                        identity=ident[:])
                    y2T = sbuf.tile([P, P], f32, tag="y2T")
                    nc.vector.tensor_scalar(
                        out=y2T[0:F, :], in0=ptt[0:F, :],
                        scalar1=sccol2[0:F, 0:1], scalar2=sccol2[0:F, 1:2],
                        op0=OP.mult, op1=OP.add)
                    pzt = psumb.tile([P, C + 2 * H], f32, space="PSUM", tag="big")
                    pz = pzt[:, 0:END]
                    mm(pz, y2T[0:F, :], Wo1_t[0:F, :], start=True, stop=False)
                    mm(pz, ones_r2[0:1, :], bo1_t[0:1, :], start=False, stop=True)
                    zr = sbuf.tile([P, END], f32, tag="zr")
                    nc.vector.tensor_scalar(out=zr[:], in0=pz[:], scalar1=0.0,
                                            scalar2=None, op0=OP.max)
                    zw = sbuf.tile([P, C], f32, tag="zw")
                    nc.vector.tensor_tensor(out=zw[:], in0=zr[:], in1=Wo2r_t[:],
                                            op=OP.mult)
                    res = sbuf.tile([P, 1], f32, tag="res")
                    nc.vector.tensor_reduce(out=res[:], in_=zw[:], axis=AX.X,
                                            op=OP.add)
                    nc.vector.tensor_tensor(out=res[:], in0=res[:], in1=bo2r_t[:],
                                            op=OP.add)
                    nc.sync.dma_start(out=d_out[t * P : (t + 1) * P, :], in_=res[:])

    nc.compile()
    return nc


def kernel(**inputs):
    X = np.asarray(inputs["X"], np.float32)
    prep = _host_prep(X, inputs["edge_index"])
    wts = _build_weights(inputs)

    key = ("prog", tuple(prep["CW"]))
    if key not in _CACHE:
        _CACHE.clear()
        _CACHE[key] = _build_program(prep["CW"], prep["woff"], prep["slots"])
    nc = _CACHE[key]

    in_maps = []
    for c in range(NCORES):
        m = dict(
            xrows=prep["xrows"][c],
            idx_tiles=prep["idx_tiles"][c],
            pm=prep["pm"][c],
        )
        m.update(wts)
        in_maps.append(m)

    from concourse.bass_utils import run_bass_kernel_spmd
    res = run_bass_kernel_spmd(nc, in_maps, list(range(NCORES)))

    out = np.zeros((N, 1), np.float32)
    for c in range(NCORES):
        pc = prep["perm"][c * NPCP : (c + 1) * NPCP]
        m = pc >= 0
        out[pc[m]] = res.results[c]["out"][m, :]
    return out



# revision 12
# speedup vs baseline: 1.4399x; 1.0469x over previous
"""GAT (2-layer, 4-head) message-passing kernel for 8 Trainium2 NeuronCores.

Sharding: nodes split into 8 contiguous ranges of 6250 (padded to 6272); within
each core nodes are sorted by in-degree into 49 windows of 128 (one dst node
per SBUF partition). Each core builds hidden-table rows (h | a_s | a_d) for its
nodes, the table is AllGathered, and each core processes its own in-edges:
edge slot (p, c) = c-th in-edge of the window's p-th node. h[src] rows are
fetched with dma_gather using int16 PAIR row indices (2x320 f32 = 2560B
descriptors); a parity mask zeroes the unused pair half. Per-edge softmax
weights ex = exp(leakyrelu(a_s[src]+a_d[dst])) multiply the messages on DVE,
and identity-weight matmuls accumulate the per-partition sums in PSUM (with ex
riding along as 4 extra columns -> softmax denominators). Normalization, head
mean, batchnorm moments (ones-matmuls + 2xC AllReduce) and the MLP head follow.
Biases b1/b2 cancel inside the following batchnorms and are dropped.
"""

import numpy as np

N = 50000
E = 800000
IN = 128
T = 8
H = 4
F = 64
C = 256
END = 256
NCORES = 8
NPC = 6250
NPCP = 6272
NW = NPCP // 128
P = 128
DW = 288              # table row: 256 h | 4 a_s | 4 a_d | 24 pad
ROWS = NCORES * NPCP
EPS = 1e-5
CAP = 10              # max slot-columns per gather chunk
COMBINE = True        # pre-combine parity slabs on Pool before the PE matmul

_CACHE = {}


def _host_prep(X, edge_index):
    ei = np.asarray(edge_index)
    src = ei[0].astype(np.int64)
    dst = ei[1].astype(np.int64)
    deg = np.bincount(dst, minlength=N)

    # global degree-desc ranking: rank r -> window r//1024, core (r%1024)//128,
    # lane r%128.  All cores share each window's degree profile, so the shared
    # per-window max (CW) tracks the true degrees tightly.
    order = np.argsort(-deg, kind="stable")
    perm = np.empty(NCORES * NPCP, np.int64)
    perm.fill(-1)
    tpos = np.empty(N, np.int64)
    r = np.arange(N)
    w_of = r // (NCORES * P)
    core_of = (r % (NCORES * P)) // P
    lane_of = r % P
    pos = core_of * NPCP + w_of * P + lane_of
    perm[pos] = order
    tpos[order] = pos

    stp = tpos[src]
    dtp = tpos[dst]
    dcore = dtp // NPCP
    dlocal = dtp % NPCP

    degs = np.zeros(NCORES * NPCP, np.int64)
    degs[tpos[np.arange(N)]] = deg
    cw = degs.reshape(NCORES, NW, P).max(axis=2)
    CW = [int(x) for x in np.maximum(cw.max(axis=0), 1)]
    woff = np.concatenate([[0], np.cumsum(np.array(CW, np.int64))])
    slots = int(woff[-1]) * P

    order = np.lexsort((stp, dtp))
    sdtp, sstp = dtp[order], stp[order]
    sdcore, sdlocal = dcore[order], dlocal[order]
    uniq, counts = np.unique(sdtp, return_counts=True)
    ranks = np.arange(E) - np.repeat(np.cumsum(counts) - counts, counts)

    w = sdlocal // P
    p = sdlocal % P
    slot = (woff[w] + ranks) * P + p

    idx_pair = np.zeros((NCORES, slots), np.int16)
    pmask = np.zeros((NCORES, slots, 2), np.float32)
    for c in range(NCORES):
        m = sdcore == c
        sl = slot[m]
        st = sstp[m]
        idx_pair[c, sl] = (st // 2).astype(np.int16)
        pmask[c, sl, 0] = (st % 2 == 0).astype(np.float32)
        pmask[c, sl, 1] = (st % 2 == 1).astype(np.float32)

    def pack16(a):
        b = a.reshape(-1, 16).T
        return np.tile(b, (8, 1))

    idx_tiles = np.stack([pack16(idx_pair[c]) for c in range(NCORES)])
    pm = pmask.reshape(NCORES, slots // P, P, 2).transpose(0, 2, 1, 3).copy()

    Xf = np.ascontiguousarray(X[:, :, T - 1]).astype(np.float32)
    xrows = np.zeros((NCORES, NPCP, IN), np.float32)
    for c in range(NCORES):
        pc = perm[c * NPCP : (c + 1) * NPCP]
        m = pc >= 0
        xrows[c, m] = Xf[pc[m]]

    return dict(CW=CW, woff=woff, slots=slots, idx_tiles=idx_tiles,
                pm=pm, perm=perm, xrows=xrows)


def _build_weights(inp):
    f32 = np.float32
    W_in = np.asarray(inp["W_in"], f32)
    W1 = np.asarray(inp["W1"], f32)
    W2 = np.asarray(inp["W2"], f32)

    def att_mat(a_s, a_d):
        A = np.zeros((C, 2 * H), f32)
        for k in range(H):
            A[64 * k : 64 * (k + 1), k] = a_s[k]
            A[64 * k : 64 * (k + 1), H + k] = a_d[k]
        return A

    WA1 = W1 @ att_mat(np.asarray(inp["as1"], f32), np.asarray(inp["ad1"], f32))
    WA2 = W2 @ att_mat(np.asarray(inp["as2"], f32), np.asarray(inp["ad2"], f32))
    b_in = np.asarray(inp["b_in"], f32)
    return dict(
        W_in=W_in,
        W1s=np.ascontiguousarray(np.stack([W1[:128], W1[128:]], axis=1)),
        W2s=np.ascontiguousarray(np.stack([W2[:128], W2[128:]], axis=1)),
        WA1s=np.ascontiguousarray(np.stack([WA1[:128], WA1[128:]], axis=1)),
        WA2s=np.ascontiguousarray(np.stack([WA2[:128], WA2[128:]], axis=1)),
        b_in_cols=np.ascontiguousarray(np.stack([b_in[:128], b_in[128:]], 1)),
        g1=np.asarray(inp["g1"], f32)[None, :],
        be1=np.asarray(inp["be1"], f32)[None, :],
        g2=np.asarray(inp["g2"], f32)[None, :],
        be2=np.asarray(inp["be2"], f32)[None, :],
        Wo1=np.asarray(inp["Wo1"], f32),
        bo1=np.asarray(inp["bo1"], f32)[None, :],
        Wo2rep=np.ascontiguousarray(
            np.broadcast_to(np.asarray(inp["Wo2"], f32)[:, 0][None, :], (P, C))),
        bo2rep=np.full((P, 1), float(np.asarray(inp["bo2"]).reshape(-1)[0]), f32),
        ident=np.eye(P, dtype=f32),
        ones=np.ones((P, 1), f32),
        ones_row=np.ones((1, P), f32),
    )


def _build_program(CW, woff, slots, repeat=1, local_cc=None):
    import concourse.bacc as bacc
    import concourse.tile as tile
    from concourse import mybir

    if local_cc is None:
        local_cc = repeat > 1
    nc = bacc.Bacc("TRN2", num_devices=NCORES)
    dt = mybir.dt
    f32 = dt.float32
    AX = mybir.AxisListType
    OP = mybir.AluOpType
    ACT = mybir.ActivationFunctionType
    CCG = [list(range(NCORES))]

    d_x = nc.declare_dram_parameter("xrows", [NPCP, IN], f32, isOutput=False)
    d_idx = nc.declare_dram_parameter("idx_tiles", [P, slots // 16], dt.int16,
                                      isOutput=False)
    d_pm = nc.declare_dram_parameter("pm", [P, slots // P, 2], f32, isOutput=False)
    d_Win = nc.declare_dram_parameter("W_in", [IN, C], f32, isOutput=False)
    d_W1s = nc.declare_dram_parameter("W1s", [P, 2, C], f32, isOutput=False)
    d_W2s = nc.declare_dram_parameter("W2s", [P, 2, C], f32, isOutput=False)
    d_WA1s = nc.declare_dram_parameter("WA1s", [P, 2, 2 * H], f32, isOutput=False)
    d_WA2s = nc.declare_dram_parameter("WA2s", [P, 2, 2 * H], f32, isOutput=False)
    d_binc = nc.declare_dram_parameter("b_in_cols", [P, 2], f32, isOutput=False)
    d_g1 = nc.declare_dram_parameter("g1", [1, C], f32, isOutput=False)
    d_be1 = nc.declare_dram_parameter("be1", [1, C], f32, isOutput=False)
    d_g2 = nc.declare_dram_parameter("g2", [1, F], f32, isOutput=False)
    d_be2 = nc.declare_dram_parameter("be2", [1, F], f32, isOutput=False)
    d_Wo1 = nc.declare_dram_parameter("Wo1", [F, END], f32, isOutput=False)
    d_bo1 = nc.declare_dram_parameter("bo1", [1, END], f32, isOutput=False)
    d_Wo2r = nc.declare_dram_parameter("Wo2rep", [P, C], f32, isOutput=False)
    d_bo2r = nc.declare_dram_parameter("bo2rep", [P, 1], f32, isOutput=False)
    d_id = nc.declare_dram_parameter("ident", [P, P], f32, isOutput=False)
    d_ones = nc.declare_dram_parameter("ones", [P, 1], f32, isOutput=False)
    d_onesr = nc.declare_dram_parameter("ones_row", [1, P], f32, isOutput=False)
    d_out = nc.declare_dram_parameter("out", [NPCP, 1], f32, isOutput=True)

    loc1 = nc.dram_tensor("loc1", [NPCP, DW], f32)
    tab1 = nc.dram_tensor("tab1", [ROWS, DW], f32, addr_space="Shared")
    g1loc = nc.dram_tensor("g1loc", [NPCP, C], f32)
    loc2 = nc.dram_tensor("loc2", [NPCP, DW], f32)
    tab2 = nc.dram_tensor("tab2", [ROWS, DW], f32, addr_space="Shared")
    g2loc = nc.dram_tensor("g2loc", [NPCP, F], f32)
    st1 = nc.dram_tensor("st1", [2, C], f32)
    st1r = nc.dram_tensor("st1r", [2, C], f32, addr_space="Shared")
    st2 = nc.dram_tensor("st2", [2, F], f32)
    st2r = nc.dram_tensor("st2r", [2, F], f32, addr_space="Shared")
    sc1 = nc.dram_tensor("sc1", [2, C], f32)
    sc2 = nc.dram_tensor("sc2", [2, F], f32)

    def mm(out, lhsT, rhs, start, stop):
        nc.tensor.matmul(out=out, lhsT=lhsT, rhs=rhs, start=start, stop=stop)

    import contextlib
    with tile.TileContext(nc) as tc:
        with (
            tc.tile_pool(name="const", bufs=1) as cpool,
            tc.tile_pool(name="sbuf", bufs=2) as sbuf,
            tc.tile_pool(name="gat", bufs=3) as gpool,
            tc.tile_pool(name="msgp", bufs=2) as mpool,
            tc.tile_pool(name="psum", bufs=2, space="PSUM") as psum,
            tc.tile_pool(name="psumb", bufs=3, space="PSUM") as psumb,
            tc.tile_pool(name="pstat", bufs=1, space="PSUM") as pstat,
        ):
            def ctile(dram, shape, tag, dtt=f32):
                t = cpool.tile(shape, dtt, tag=tag)
                nc.sync.dma_start(out=t[:], in_=dram[:])
                return t

            ident = ctile(d_id, [P, P], "ident")
            ones = ctile(d_ones, [P, 1], "ones")
            ones_r2 = cpool.tile([P, P], f32, tag="ones_r")
            nc.sync.dma_start(out=ones_r2[0:1, :], in_=d_onesr[:])
            Win_t = ctile(d_Win, [IN, C], "Win")
            W1_t = ctile(d_W1s, [P, 2, C], "W1")
            W2_t = ctile(d_W2s, [P, 2, C], "W2")
            WA1_t = ctile(d_WA1s, [P, 2, 2 * H], "WA1")
            WA2_t = ctile(d_WA2s, [P, 2, 2 * H], "WA2")
            binc_t = ctile(d_binc, [P, 2], "binc")
            Wo1_t = cpool.tile([P, END], f32, tag="Wo1")
            nc.sync.dma_start(out=Wo1_t[0:F, :], in_=d_Wo1[:])
            bo1_t = cpool.tile([P, END], f32, tag="bo1")
            nc.sync.dma_start(out=bo1_t[0:1, :], in_=d_bo1[:])
            Wo2r_t = ctile(d_Wo2r, [P, C], "Wo2r")
            bo2r_t = ctile(d_bo2r, [P, 1], "bo2r")
            idx_t = ctile(d_idx, [P, slots // 16], "idxt", dt.int16)
            pm_t = ctile(d_pm, [P, slots // P, 2], "pmt")

            rep_cm = tc.For_i(0, repeat, 1) if repeat > 1 else contextlib.nullcontext()
            with rep_cm:
                # ---------------- table-row builder -------------------------
                def build_table(rows_getter, W_t, WA_t, loc):
                    for t in range(NW):
                        yT = rows_getter(t)
                        ph = psumb.tile([P, C + 2 * H], f32, space="PSUM", tag="big")
                        for hf in range(2):
                            mm(ph[:, 0:C], yT[hf][:], W_t[:, hf, :],
                               start=(hf == 0), stop=(hf == 1))
                            nc.tensor.matmul(out=ph[:, C : C + 2 * H],
                                             lhsT=yT[hf][:], rhs=WA_t[:, hf, :],
                                             start=False, stop=False,
                                             skip_group_check=True)
                        stg = sbuf.tile([P, DW], f32, tag="stgA")
                        nc.vector.tensor_copy(out=stg[:, 0 : C + 2 * H],
                                              in_=ph[:, 0 : C + 2 * H])
                        nc.vector.memset(stg[:, C + 2 * H : DW], 0.0)
                        nc.sync.dma_start(out=loc[t * P : (t + 1) * P, :], in_=stg[:])

                # ---------------- phase A ------------------------------------
                def phaseA_rows(t):
                    xs = sbuf.tile([P, IN], f32, tag="xs")
                    nc.sync.dma_start(out=xs[:], in_=d_x[t * P : (t + 1) * P, :])
                    pt = psum.tile([P, P], f32, space="PSUM", tag="tr")
                    nc.tensor.transpose(out=pt[:], in_=xs[:], identity=ident[:])
                    xsT = sbuf.tile([P, P], f32, tag="xsT")
                    nc.vector.tensor_copy(out=xsT[:], in_=pt[:])
                    yT = []
                    for hf in range(2):
                        px = psum.tile([P, P], f32, space="PSUM", tag="tr")
                        mm(px[:], Win_t[:, hf * P : (hf + 1) * P], xsT[:],
                           start=True, stop=True)
                        xt = sbuf.tile([P, P], f32, tag=f"x0T{hf}")
                        nc.vector.tensor_tensor(
                            out=xt[:], in0=px[:],
                            in1=binc_t[:, hf : hf + 1].broadcast_to([P, P]),
                            op=OP.add)
                        yT.append(xt)
                    return yT

                build_table(phaseA_rows, W1_t, WA1_t, loc1)
                if local_cc:
                    nc.sync.dma_start(out=tab1[0:NPCP, :], in_=loc1[:])
                else:
                    nc.gpsimd.collective_compute(
                        "AllGather", OP.bypass, replica_groups=CCG,
                        ins=[loc1[:].opt()], outs=[tab1[:].opt()])

                # ---------------- edge phase ---------------------------------
                def edge_phase(tab, loc, layer):
                    outw = C if layer == 1 else F
                    pstats = pstat.tile([P, C], f32, space="PSUM", tag="sx")
                    pstats2 = pstat.tile([P, C], f32, space="PSUM", tag="sxx")
                    tabv = tab[:].rearrange("(q two) d -> q (two d)", two=2)

                    accs = cpool.tile([P, C], f32, tag=f"accs{layer}")
                    accq = cpool.tile([P, C], f32, tag=f"accq{layer}")
                    nc.vector.memset(accs[:, 0:outw], 0.0)
                    nc.vector.memset(accq[:, 0:outw], 0.0)

                    def flush(w, po):
                        sden = sbuf.tile([P, H], f32, tag="sden")
                        nc.vector.tensor_scalar(out=sden[:], in0=po[:, C : C + H],
                                                scalar1=1e-16, scalar2=None,
                                                op0=OP.add)
                        rs = sbuf.tile([P, H], f32, tag="rs")
                        nc.vector.reciprocal(out=rs[:], in_=sden[:])
                        if layer == 1:
                            org = sbuf.tile([P, C], f32, tag="org")
                            nc.vector.tensor_tensor(
                                out=org[:].rearrange("p (k f) -> p k f", k=H),
                                in0=po[:, 0:C].rearrange("p (k f) -> p k f", k=H),
                                in1=rs[:].unsqueeze(2).broadcast_to([P, H, F]),
                                op=OP.mult)
                            nc.sync.dma_start(out=g1loc[w * P : (w + 1) * P, :],
                                              in_=org[:])
                            _acc_stats(org[:])
                        else:
                            nc.vector.tensor_scalar(out=rs[:], in0=rs[:],
                                                    scalar1=0.25, scalar2=None,
                                                    op0=OP.mult)
                            tmp = sbuf.tile([P, C], f32, tag="tmp2")
                            nc.vector.tensor_tensor(
                                out=tmp[:].rearrange("p (k f) -> p k f", k=H),
                                in0=po[:, 0:C].rearrange("p (k f) -> p k f", k=H),
                                in1=rs[:].unsqueeze(2).broadcast_to([P, H, F]),
                                op=OP.mult)
                            org = sbuf.tile([P, F], f32, tag="orgf")
                            nc.vector.tensor_tensor(out=org[:], in0=tmp[:, 0:F],
                                                    in1=tmp[:, F : 2 * F], op=OP.add)
                            nc.vector.tensor_tensor(out=org[:], in0=org[:],
                                                    in1=tmp[:, 2 * F : 3 * F],
                                                    op=OP.add)
                            nc.vector.tensor_tensor(out=org[:], in0=org[:],
                                                    in1=tmp[:, 3 * F : 4 * F],
                                                    op=OP.add)
                            nc.sync.dma_start(out=g2loc[w * P : (w + 1) * P, :],
                                              in_=org[:])
                            _acc_stats(org[:])

                    def _acc_stats(orgap):
                        nc.vector.tensor_tensor(out=accs[:, 0:outw],
                                                in0=accs[:, 0:outw], in1=orgap,
                                                op=OP.add)
                        sq = sbuf.tile([P, C], f32, tag="sq")
                        nc.vector.tensor_tensor(out=sq[:, 0:outw], in0=orgap,
                                                in1=orgap, op=OP.mult)
                        nc.vector.tensor_tensor(out=accq[:, 0:outw],
                                                in0=accq[:, 0:outw],
                                                in1=sq[:, 0:outw], op=OP.add)

                    pend = []
                    for w in range(NW):
                        cw = CW[w]
                        off = int(woff[w])
                        attD = sbuf.tile([P, H], f32, tag="attD")
                        nc.sync.dma_start(
                            out=attD[:],
                            in_=loc[w * P : (w + 1) * P, C + H : C + 2 * H])
                        po = psumb.tile([P, C + 2 * H], f32, space="PSUM",
                                        tag="big")
                        nsub = (cw + CAP - 1) // CAP
                        szs = [cw // nsub + (1 if i < cw % nsub else 0)
                               for i in range(nsub)]
                        offs = [sum(szs[:i]) for i in range(nsub)]
                        for s in range(nsub):
                            c0 = offs[s]
                            ns = szs[s]
                            hg = gpool.tile([P, CAP, 2 * DW], f32, tag="hg")
                            nc.gpsimd.dma_gather(
                                out_ap=hg[:, 0:ns, :],
                                in_ap=tabv,
                                idxs_ap=idx_t[:, (off + c0) * 8 : (off + c0 + ns) * 8],
                                num_idxs=ns * P,
                                num_idxs_reg=ns * P,
                                elem_size=2 * DW,
                                single_packet=False,
                            )
                            hgv = hg[:, 0:ns, :].rearrange(
                                "p c (two d) -> p c two d", two=2)
                            ex = mpool.tile([P, CAP, 2, H], f32, tag="ex")
                            nc.vector.tensor_tensor(
                                out=ex[:, 0:ns],
                                in0=hgv[:, :, :, C : C + H],
                                in1=attD[:].unsqueeze(1).unsqueeze(1)
                                    .broadcast_to([P, ns, 2, H]),
                                op=OP.add)
                            lr = mpool.tile([P, CAP, 2, H], f32, tag="lr")
                            nc.vector.tensor_scalar(
                                out=lr[:, 0:ns], in0=ex[:, 0:ns], scalar1=0.2,
                                scalar2=None, op0=OP.mult)
                            nc.vector.tensor_tensor(
                                out=lr[:, 0:ns], in0=lr[:, 0:ns], in1=ex[:, 0:ns],
                                op=OP.max)
                            nc.scalar.activation(out=ex[:, 0:ns], in_=lr[:, 0:ns],
                                                 func=ACT.Exp)
                            nc.vector.tensor_tensor(
                                out=hgv[:, :, :, C : C + H],
                                in0=ex[:, 0:ns],
                                in1=pm_t[:, off + c0 : off + c0 + ns, :]
                                    .unsqueeze(3).broadcast_to([P, ns, 2, H]),
                                op=OP.mult)
                            for par in range(2):
                                nc.vector.tensor_tensor(
                                    out=hgv[:, :, par, 0:C].rearrange(
                                        "p c (k f) -> p c k f", k=H),
                                    in0=hgv[:, :, par, 0:C].rearrange(
                                        "p c (k f) -> p c k f", k=H),
                                    in1=hgv[:, :, par, C : C + H].unsqueeze(3)
                                        .broadcast_to([P, ns, H, F]),
                                    op=OP.mult)
                            for par in range(2):
                                for cc in range(ns):
                                    mm(po[:, 0 : C + H], ident[:],
                                       hgv[:, cc, par, 0 : C + H],
                                       start=(s == 0 and par == 0 and cc == 0),
                                       stop=(s == nsub - 1 and par == 1
                                             and cc == ns - 1))
                        pend.append((w, po))
                        if len(pend) == 2:
                            flush(*pend.pop(0))
                    for item in pend:
                        flush(*item)
                    mm(pstats[0:1, 0:outw], ones[:], accs[:, 0:outw],
                       start=True, stop=True)
                    mm(pstats2[0:1, 0:outw], ones[:], accq[:, 0:outw],
                       start=True, stop=True)
                    # moments -> AllReduce -> scale/shift rows in DRAM
                    stg0 = sbuf.tile([P, C], f32, tag="stg0")
                    nc.vector.tensor_copy(out=stg0[0:1, 0:outw],
                                          in_=pstats[0:1, 0:outw])
                    stg1 = sbuf.tile([P, C], f32, tag="stg1")
                    nc.vector.tensor_copy(out=stg1[0:1, 0:outw],
                                          in_=pstats2[0:1, 0:outw])
                    std = st1 if layer == 1 else st2
                    stdr = st1r if layer == 1 else st2r
                    nc.sync.dma_start(out=std[0:1, :], in_=stg0[0:1, 0:outw])
                    nc.sync.dma_start(out=std[1:2, :], in_=stg1[0:1, 0:outw])
                    if local_cc:
                        nc.sync.dma_start(out=stdr[:, :], in_=std[:])
                    else:
                        nc.gpsimd.collective_compute(
                            "AllReduce", OP.add, replica_groups=CCG,
                            ins=[std[:].opt()], outs=[stdr[:].opt()])
                    # single-partition workspace: slices share one partition
                    bn = cpool.tile([1, 10 * C], f32, tag="bn")
                    r0 = bn[:, 0 * C : 0 * C + outw]
                    r1 = bn[:, 1 * C : 1 * C + outw]
                    gv = bn[:, 2 * C : 2 * C + outw]
                    bev = bn[:, 3 * C : 3 * C + outw]
                    mu = bn[:, 4 * C : 4 * C + outw]
                    var = bn[:, 5 * C : 5 * C + outw]
                    msq = bn[:, 6 * C : 6 * C + outw]
                    rstd = bn[:, 7 * C : 7 * C + outw]
                    scl = bn[:, 8 * C : 8 * C + outw]
                    shf = bn[:, 9 * C : 9 * C + outw]
                    nc.sync.dma_start(out=r0, in_=stdr[0:1, :])
                    nc.sync.dma_start(out=r1, in_=stdr[1:2, :])
                    nc.sync.dma_start(out=gv, in_=(d_g1 if layer == 1 else d_g2)[:])
                    nc.sync.dma_start(out=bev, in_=(d_be1 if layer == 1 else d_be2)[:])
                    nc.vector.tensor_scalar(out=mu, in0=r0, scalar1=1.0 / N,
                                            scalar2=None, op0=OP.mult)
                    nc.vector.tensor_scalar(out=var, in0=r1, scalar1=1.0 / N,
                                            scalar2=None, op0=OP.mult)
                    nc.vector.tensor_tensor(out=msq, in0=mu, in1=mu, op=OP.mult)
                    nc.vector.tensor_tensor(out=var, in0=var, in1=msq, op=OP.subtract)
                    nc.vector.tensor_scalar(out=var, in0=var, scalar1=EPS,
                                            scalar2=None, op0=OP.add)
                    nc.scalar.activation(out=msq, in_=var, func=ACT.Sqrt)
                    nc.vector.reciprocal(out=rstd, in_=msq)
                    nc.vector.tensor_tensor(out=scl, in0=gv, in1=rstd, op=OP.mult)
                    nc.vector.tensor_tensor(out=shf, in0=mu, in1=scl, op=OP.mult)
                    nc.vector.tensor_tensor(out=shf, in0=bev, in1=shf, op=OP.subtract)
                    scd = sc1 if layer == 1 else sc2
                    nc.sync.dma_start(out=scd[0:1, :], in_=scl)
                    nc.sync.dma_start(out=scd[1:2, :], in_=shf)

                edge_phase(tab1, loc1, 1)

                # ---------------- phase E ------------------------------------
                sccol1 = sbuf.tile([P, 4], f32, tag="sccol1")
                nc.sync.dma_start(
                    out=sccol1[:].rearrange("p (r h) -> p r h", r=2),
                    in_=sc1[:].rearrange("r (h p) -> p r h", p=P))

                def phaseE_rows(t):
                    g1r = sbuf.tile([P, C], f32, tag="g1r")
                    nc.sync.dma_start(out=g1r[:], in_=g1loc[t * P : (t + 1) * P, :])
                    yT = []
                    for hf in range(2):
                        ptt = psum.tile([P, P], f32, space="PSUM", tag="tr")
                        nc.tensor.transpose(out=ptt[:],
                                            in_=g1r[:, hf * P : (hf + 1) * P],
                                            identity=ident[:])
                        yt = sbuf.tile([P, P], f32, tag=f"yT{hf}")
                        nc.vector.tensor_scalar(
                            out=yt[:], in0=ptt[:],
                            scalar1=sccol1[:, hf : hf + 1],
                            scalar2=sccol1[:, 2 + hf : 3 + hf],
                            op0=OP.mult, op1=OP.add)
                        nc.vector.tensor_scalar(out=yt[:], in0=yt[:], scalar1=0.0,
                                                scalar2=None, op0=OP.max)
                        yT.append(yt)
                    return yT

                build_table(phaseE_rows, W2_t, WA2_t, loc2)
                if local_cc:
                    nc.sync.dma_start(out=tab2[0:NPCP, :], in_=loc2[:])
                else:
                    nc.gpsimd.collective_compute(
                        "AllGather", OP.bypass, replica_groups=CCG,
                        ins=[loc2[:].opt()], outs=[tab2[:].opt()])

                edge_phase(tab2, loc2, 2)

                # ---------------- phase I ------------------------------------
                sccol2 = sbuf.tile([P, 2], f32, tag="sccol2")
                nc.sync.dma_start(out=sccol2[0:F, :],
                                  in_=sc2[:].rearrange("r f -> f r"))
                for t in range(NW):
                    g2r = sbuf.tile([P, F], f32, tag="g2r")
                    nc.sync.dma_start(out=g2r[:], in_=g2loc[t * P : (t + 1) * P, :])
                    ptt = psum.tile([P, P], f32, space="PSUM", tag="tr")
                    nc.tensor.transpose(out=ptt[0:F, :], in_=g2r[:],
                                        identity=ident[:])
                    y2T = sbuf.tile([P, P], f32, tag="y2T")
                    nc.vector.tensor_scalar(
                        out=y2T[0:F, :], in0=ptt[0:F, :],
                        scalar1=sccol2[0:F, 0:1], scalar2=sccol2[0:F, 1:2],
                        op0=OP.mult, op1=OP.add)
                    pzt = psumb.tile([P, C + 2 * H], f32, space="PSUM", tag="big")
                    pz = pzt[:, 0:END]
                    mm(pz, y2T[0:F, :], Wo1_t[0:F, :], start=True, stop=False)
                    mm(pz, ones_r2[0:1, :], bo1_t[0:1, :], start=False, stop=True)
                    zr = sbuf.tile([P, END], f32, tag="zr")
                    nc.vector.tensor_scalar(out=zr[:], in0=pz[:], scalar1=0.0,
                                            scalar2=None, op0=OP.max)
                    zw = sbuf.tile([P, C], f32, tag="zw")
                    nc.vector.tensor_tensor(out=zw[:], in0=zr[:], in1=Wo2r_t[:],
                                            op=OP.mult)
                    res = sbuf.tile([P, 1], f32, tag="res")
                    nc.vector.tensor_reduce(out=res[:], in_=zw[:], axis=AX.X,
                                            op=OP.add)
                    nc.vector.tensor_tensor(out=res[:], in0=res[:], in1=bo2r_t[:],
                                            op=OP.add)
                    nc.sync.dma_start(out=d_out[t * P : (t + 1) * P, :], in_=res[:])

    nc.compile()
    return nc


def kernel(**inputs):
    X = np.asarray(inputs["X"], np.float32)
    prep = _host_prep(X, inputs["edge_index"])
    wts = _build_weights(inputs)

    key = ("prog", tuple(prep["CW"]))
    if key not in _CACHE:
        _CACHE.clear()
        _CACHE[key] = _build_program(prep["CW"], prep["woff"], prep["slots"])
    nc = _CACHE[key]

    in_maps = []
    for c in range(NCORES):
        m = dict(
            xrows=prep["xrows"][c],
            idx_tiles=prep["idx_tiles"][c],
            pm=prep["pm"][c],
        )
        m.update(wts)
        in_maps.append(m)

    from concourse.bass_utils import run_bass_kernel_spmd
    res = run_bass_kernel_spmd(nc, in_maps, list(range(NCORES)))

    out = np.zeros((N, 1), np.float32)
    for c in range(NCORES):
        pc = prep["perm"][c * NPCP : (c + 1) * NPCP]
        m = pc >= 0
        out[pc[m]] = res.results[c]["out"][m, :]
    return out



# revision 13
# speedup vs baseline: 1.5082x; 1.0474x over previous
"""GAT (2-layer, 4-head) message-passing kernel for 8 Trainium2 NeuronCores.

Sharding: nodes split into 8 contiguous ranges of 6250 (padded to 6272); within
each core nodes are sorted by in-degree into 49 windows of 128 (one dst node
per SBUF partition). Each core builds hidden-table rows (h | a_s | a_d) for its
nodes, the table is AllGathered, and each core processes its own in-edges:
edge slot (p, c) = c-th in-edge of the window's p-th node. h[src] rows are
fetched with dma_gather using int16 PAIR row indices (2x320 f32 = 2560B
descriptors); a parity mask zeroes the unused pair half. Per-edge softmax
weights ex = exp(leakyrelu(a_s[src]+a_d[dst])) multiply the messages on DVE,
and identity-weight matmuls accumulate the per-partition sums in PSUM (with ex
riding along as 4 extra columns -> softmax denominators). Normalization, head
mean, batchnorm moments (ones-matmuls + 2xC AllReduce) and the MLP head follow.
Biases b1/b2 cancel inside the following batchnorms and are dropped.
"""

import numpy as np

N = 50000
E = 800000
IN = 128
T = 8
H = 4
F = 64
C = 256
END = 256
NCORES = 8
NPC = 6250
NPCP = 6272
NW = NPCP // 128
P = 128
DW = 288              # table row: 256 h | 4 a_s | 4 a_d | 24 pad
ROWS = NCORES * NPCP
EPS = 1e-5
CAP = 10              # max slot-columns per gather chunk
COMBINE = True        # pre-combine parity slabs on Pool before the PE matmul

_CACHE = {}


def _host_prep(X, edge_index):
    ei = np.asarray(edge_index)
    src = ei[0].astype(np.int64)
    dst = ei[1].astype(np.int64)
    deg = np.bincount(dst, minlength=N)

    # global degree-desc ranking: rank r -> window r//1024, core (r%1024)//128,
    # lane r%128.  All cores share each window's degree profile, so the shared
    # per-window max (CW) tracks the true degrees tightly.
    order = np.argsort(-deg, kind="stable")
    perm = np.empty(NCORES * NPCP, np.int64)
    perm.fill(-1)
    tpos = np.empty(N, np.int64)
    r = np.arange(N)
    w_of = r // (NCORES * P)
    core_of = (r % (NCORES * P)) // P
    lane_of = r % P
    pos = core_of * NPCP + w_of * P + lane_of
    perm[pos] = order
    tpos[order] = pos

    stp = tpos[src]
    dtp = tpos[dst]
    dcore = dtp // NPCP
    dlocal = dtp % NPCP

    degs = np.zeros(NCORES * NPCP, np.int64)
    degs[tpos[np.arange(N)]] = deg
    cw = degs.reshape(NCORES, NW, P).max(axis=2)
    CW = [int(x) for x in np.maximum(cw.max(axis=0), 1)]
    woff = np.concatenate([[0], np.cumsum(np.array(CW, np.int64))])
    slots = int(woff[-1]) * P

    order = np.lexsort((stp, dtp))
    sdtp, sstp = dtp[order], stp[order]
    sdcore, sdlocal = dcore[order], dlocal[order]
    uniq, counts = np.unique(sdtp, return_counts=True)
    ranks = np.arange(E) - np.repeat(np.cumsum(counts) - counts, counts)

    w = sdlocal // P
    p = sdlocal % P
    slot = (woff[w] + ranks) * P + p

    idx_pair = np.zeros((NCORES, slots), np.int16)
    pmask = np.zeros((NCORES, slots, 2), np.float32)
    for c in range(NCORES):
        m = sdcore == c
        sl = slot[m]
        st = sstp[m]
        idx_pair[c, sl] = (st // 2).astype(np.int16)
        pmask[c, sl, 0] = (st % 2 == 0).astype(np.float32)
        pmask[c, sl, 1] = (st % 2 == 1).astype(np.float32)

    def pack16(a):
        b = a.reshape(-1, 16).T
        return np.tile(b, (8, 1))

    idx_tiles = np.stack([pack16(idx_pair[c]) for c in range(NCORES)])
    pm = pmask.reshape(NCORES, slots // P, P, 2).transpose(0, 2, 1, 3).copy()

    Xf = np.ascontiguousarray(X[:, :, T - 1]).astype(np.float32)
    xrows = np.zeros((NCORES, NPCP, IN), np.float32)
    for c in range(NCORES):
        pc = perm[c * NPCP : (c + 1) * NPCP]
        m = pc >= 0
        xrows[c, m] = Xf[pc[m]]

    return dict(CW=CW, woff=woff, slots=slots, idx_tiles=idx_tiles,
                pm=pm, perm=perm, xrows=xrows)


def _build_weights(inp):
    f32 = np.float32
    W_in = np.asarray(inp["W_in"], f32)
    W1 = np.asarray(inp["W1"], f32)
    W2 = np.asarray(inp["W2"], f32)

    def att_mat(a_s, a_d):
        A = np.zeros((C, 2 * H), f32)
        for k in range(H):
            A[64 * k : 64 * (k + 1), k] = a_s[k]
            A[64 * k : 64 * (k + 1), H + k] = a_d[k]
        return A

    WA1 = W1 @ att_mat(np.asarray(inp["as1"], f32), np.asarray(inp["ad1"], f32))
    WA2 = W2 @ att_mat(np.asarray(inp["as2"], f32), np.asarray(inp["ad2"], f32))
    b_in = np.asarray(inp["b_in"], f32)
    return dict(
        W_in=W_in,
        W1s=np.ascontiguousarray(np.stack([W1[:128], W1[128:]], axis=1)),
        W2s=np.ascontiguousarray(np.stack([W2[:128], W2[128:]], axis=1)),
        WA1s=np.ascontiguousarray(np.stack([WA1[:128], WA1[128:]], axis=1)),
        WA2s=np.ascontiguousarray(np.stack([WA2[:128], WA2[128:]], axis=1)),
        b_in_cols=np.ascontiguousarray(np.stack([b_in[:128], b_in[128:]], 1)),
        g1=np.asarray(inp["g1"], f32)[None, :],
        be1=np.asarray(inp["be1"], f32)[None, :],
        g2=np.asarray(inp["g2"], f32)[None, :],
        be2=np.asarray(inp["be2"], f32)[None, :],
        Wo1=np.asarray(inp["Wo1"], f32),
        bo1=np.asarray(inp["bo1"], f32)[None, :],
        Wo2rep=np.ascontiguousarray(
            np.broadcast_to(np.asarray(inp["Wo2"], f32)[:, 0][None, :], (P, C))),
        bo2rep=np.full((P, 1), float(np.asarray(inp["bo2"]).reshape(-1)[0]), f32),
        ident=np.eye(P, dtype=f32),
        ones=np.ones((P, 1), f32),
        ones_row=np.ones((1, P), f32),
    )


def _build_program(CW, woff, slots, repeat=1, local_cc=None):
    import concourse.bacc as bacc
    import concourse.tile as tile
    from concourse import mybir

    if local_cc is None:
        local_cc = repeat > 1
    nc = bacc.Bacc("TRN2", num_devices=NCORES)
    dt = mybir.dt
    f32 = dt.float32
    AX = mybir.AxisListType
    OP = mybir.AluOpType
    ACT = mybir.ActivationFunctionType
    CCG = [list(range(NCORES))]

    d_x = nc.declare_dram_parameter("xrows", [NPCP, IN], f32, isOutput=False)
    d_idx = nc.declare_dram_parameter("idx_tiles", [P, slots // 16], dt.int16,
                                      isOutput=False)
    d_pm = nc.declare_dram_parameter("pm", [P, slots // P, 2], f32, isOutput=False)
    d_Win = nc.declare_dram_parameter("W_in", [IN, C], f32, isOutput=False)
    d_W1s = nc.declare_dram_parameter("W1s", [P, 2, C], f32, isOutput=False)
    d_W2s = nc.declare_dram_parameter("W2s", [P, 2, C], f32, isOutput=False)
    d_WA1s = nc.declare_dram_parameter("WA1s", [P, 2, 2 * H], f32, isOutput=False)
    d_WA2s = nc.declare_dram_parameter("WA2s", [P, 2, 2 * H], f32, isOutput=False)
    d_binc = nc.declare_dram_parameter("b_in_cols", [P, 2], f32, isOutput=False)
    d_g1 = nc.declare_dram_parameter("g1", [1, C], f32, isOutput=False)
    d_be1 = nc.declare_dram_parameter("be1", [1, C], f32, isOutput=False)
    d_g2 = nc.declare_dram_parameter("g2", [1, F], f32, isOutput=False)
    d_be2 = nc.declare_dram_parameter("be2", [1, F], f32, isOutput=False)
    d_Wo1 = nc.declare_dram_parameter("Wo1", [F, END], f32, isOutput=False)
    d_bo1 = nc.declare_dram_parameter("bo1", [1, END], f32, isOutput=False)
    d_Wo2r = nc.declare_dram_parameter("Wo2rep", [P, C], f32, isOutput=False)
    d_bo2r = nc.declare_dram_parameter("bo2rep", [P, 1], f32, isOutput=False)
    d_id = nc.declare_dram_parameter("ident", [P, P], f32, isOutput=False)
    d_ones = nc.declare_dram_parameter("ones", [P, 1], f32, isOutput=False)
    d_onesr = nc.declare_dram_parameter("ones_row", [1, P], f32, isOutput=False)
    d_out = nc.declare_dram_parameter("out", [NPCP, 1], f32, isOutput=True)

    loc1 = nc.dram_tensor("loc1", [NPCP, DW], f32)
    tab1 = nc.dram_tensor("tab1", [ROWS, DW], f32, addr_space="Shared")
    loc2 = nc.dram_tensor("loc2", [NPCP, DW], f32)
    tab2 = nc.dram_tensor("tab2", [ROWS, DW], f32, addr_space="Shared")
    st1 = nc.dram_tensor("st1", [2, C], f32)
    st1r = nc.dram_tensor("st1r", [2, C], f32, addr_space="Shared")
    st2 = nc.dram_tensor("st2", [2, F], f32)
    st2r = nc.dram_tensor("st2r", [2, F], f32, addr_space="Shared")
    sc1 = nc.dram_tensor("sc1", [2, C], f32)
    sc2 = nc.dram_tensor("sc2", [2, F], f32)

    def mm(out, lhsT, rhs, start, stop):
        nc.tensor.matmul(out=out, lhsT=lhsT, rhs=rhs, start=start, stop=stop)

    import contextlib
    with tile.TileContext(nc) as tc:
        with (
            tc.tile_pool(name="const", bufs=1) as cpool,
            tc.tile_pool(name="sbuf", bufs=2) as sbuf,
            tc.tile_pool(name="gat", bufs=3) as gpool,
            tc.tile_pool(name="msgp", bufs=2) as mpool,
            tc.tile_pool(name="psum", bufs=2, space="PSUM") as psum,
            tc.tile_pool(name="psumb", bufs=3, space="PSUM") as psumb,
            tc.tile_pool(name="pstat", bufs=1, space="PSUM") as pstat,
        ):
            def ctile(dram, shape, tag, dtt=f32):
                t = cpool.tile(shape, dtt, tag=tag)
                nc.sync.dma_start(out=t[:], in_=dram[:])
                return t

            ident = ctile(d_id, [P, P], "ident")
            ones = ctile(d_ones, [P, 1], "ones")
            ones_r2 = cpool.tile([P, P], f32, tag="ones_r")
            nc.sync.dma_start(out=ones_r2[0:1, :], in_=d_onesr[:])
            Win_t = ctile(d_Win, [IN, C], "Win")
            W1_t = ctile(d_W1s, [P, 2, C], "W1")
            W2_t = ctile(d_W2s, [P, 2, C], "W2")
            WA1_t = ctile(d_WA1s, [P, 2, 2 * H], "WA1")
            WA2_t = ctile(d_WA2s, [P, 2, 2 * H], "WA2")
            binc_t = ctile(d_binc, [P, 2], "binc")
            Wo1_t = cpool.tile([P, END], f32, tag="Wo1")
            nc.sync.dma_start(out=Wo1_t[0:F, :], in_=d_Wo1[:])
            bo1_t = cpool.tile([P, END], f32, tag="bo1")
            nc.sync.dma_start(out=bo1_t[0:1, :], in_=d_bo1[:])
            Wo2r_t = ctile(d_Wo2r, [P, C], "Wo2r")
            bo2r_t = ctile(d_bo2r, [P, 1], "bo2r")
            idx_t = ctile(d_idx, [P, slots // 16], "idxt", dt.int16)
            pm_t = ctile(d_pm, [P, slots // P, 2], "pmt")
            org1_all = cpool.tile([P, NW, C], f32, tag="org1all")
            org2_all = cpool.tile([P, NW, F], f32, tag="org2all")
            res_all = cpool.tile([P, NW], f32, tag="resall")

            rep_cm = tc.For_i(0, repeat, 1) if repeat > 1 else contextlib.nullcontext()
            with rep_cm:
                # ---------------- table-row builder -------------------------
                def build_table(rows_getter, W_t, WA_t, loc):
                    for t in range(NW):
                        yT = rows_getter(t)
                        ph = psumb.tile([P, C + 2 * H], f32, space="PSUM", tag="big")
                        for hf in range(2):
                            mm(ph[:, 0:C], yT[hf][:], W_t[:, hf, :],
                               start=(hf == 0), stop=(hf == 1))
                            nc.tensor.matmul(out=ph[:, C : C + 2 * H],
                                             lhsT=yT[hf][:], rhs=WA_t[:, hf, :],
                                             start=False, stop=False,
                                             skip_group_check=True)
                        stg = sbuf.tile([P, DW], f32, tag="stgA")
                        nc.vector.tensor_copy(out=stg[:, 0 : C + 2 * H],
                                              in_=ph[:, 0 : C + 2 * H])
                        nc.vector.memset(stg[:, C + 2 * H : DW], 0.0)
                        nc.sync.dma_start(out=loc[t * P : (t + 1) * P, :], in_=stg[:])

                # ---------------- phase A ------------------------------------
                def phaseA_rows(t):
                    xs = sbuf.tile([P, IN], f32, tag="xs")
                    nc.sync.dma_start(out=xs[:], in_=d_x[t * P : (t + 1) * P, :])
                    pt = psum.tile([P, P], f32, space="PSUM", tag="tr")
                    nc.tensor.transpose(out=pt[:], in_=xs[:], identity=ident[:])
                    xsT = sbuf.tile([P, P], f32, tag="xsT")
                    nc.vector.tensor_copy(out=xsT[:], in_=pt[:])
                    yT = []
                    for hf in range(2):
                        px = psum.tile([P, P], f32, space="PSUM", tag="tr")
                        mm(px[:], Win_t[:, hf * P : (hf + 1) * P], xsT[:],
                           start=True, stop=True)
                        xt = sbuf.tile([P, P], f32, tag=f"x0T{hf}")
                        nc.vector.tensor_tensor(
                            out=xt[:], in0=px[:],
                            in1=binc_t[:, hf : hf + 1].broadcast_to([P, P]),
                            op=OP.add)
                        yT.append(xt)
                    return yT

                build_table(phaseA_rows, W1_t, WA1_t, loc1)
                if local_cc:
                    nc.sync.dma_start(out=tab1[0:NPCP, :], in_=loc1[:])
                else:
                    nc.gpsimd.collective_compute(
                        "AllGather", OP.bypass, replica_groups=CCG,
                        ins=[loc1[:].opt()], outs=[tab1[:].opt()])

                # ---------------- edge phase ---------------------------------
                def edge_phase(tab, loc, layer):
                    outw = C if layer == 1 else F
                    pstats = pstat.tile([P, C], f32, space="PSUM", tag="sx")
                    pstats2 = pstat.tile([P, C], f32, space="PSUM", tag="sxx")
                    tabv = tab[:].rearrange("(q two) d -> q (two d)", two=2)

                    org_all = (org1_all if layer == 1 else org2_all)
                    accs = cpool.tile([P, C], f32, tag=f"accs{layer}")
                    accq = cpool.tile([P, C], f32, tag=f"accq{layer}")
                    nc.vector.memset(accs[:, 0:outw], 0.0)
                    nc.vector.memset(accq[:, 0:outw], 0.0)

                    def flush(w, po):
                        sden = sbuf.tile([P, H], f32, tag="sden")
                        nc.vector.tensor_scalar(out=sden[:], in0=po[:, C : C + H],
                                                scalar1=1e-16, scalar2=None,
                                                op0=OP.add)
                        rs = sbuf.tile([P, H], f32, tag="rs")
                        nc.vector.reciprocal(out=rs[:], in_=sden[:])
                        if layer == 1:
                            org = org_all[:, w, :]
                            nc.vector.tensor_tensor(
                                out=org.rearrange("p (k f) -> p k f", k=H),
                                in0=po[:, 0:C].rearrange("p (k f) -> p k f", k=H),
                                in1=rs[:].unsqueeze(2).broadcast_to([P, H, F]),
                                op=OP.mult)
                            _acc_stats(org)
                        else:
                            nc.vector.tensor_scalar(out=rs[:], in0=rs[:],
                                                    scalar1=0.25, scalar2=None,
                                                    op0=OP.mult)
                            tmp = sbuf.tile([P, C], f32, tag="tmp2")
                            nc.vector.tensor_tensor(
                                out=tmp[:].rearrange("p (k f) -> p k f", k=H),
                                in0=po[:, 0:C].rearrange("p (k f) -> p k f", k=H),
                                in1=rs[:].unsqueeze(2).broadcast_to([P, H, F]),
                                op=OP.mult)
                            org = org_all[:, w, :]
                            nc.vector.tensor_tensor(out=org, in0=tmp[:, 0:F],
                                                    in1=tmp[:, F : 2 * F], op=OP.add)
                            nc.vector.tensor_tensor(out=org, in0=org,
                                                    in1=tmp[:, 2 * F : 3 * F],
                                                    op=OP.add)
                            nc.vector.tensor_tensor(out=org, in0=org,
                                                    in1=tmp[:, 3 * F : 4 * F],
                                                    op=OP.add)
                            _acc_stats(org)

                    def _acc_stats(orgap):
                        nc.vector.tensor_tensor(out=accs[:, 0:outw],
                                                in0=accs[:, 0:outw], in1=orgap,
                                                op=OP.add)
                        sq = sbuf.tile([P, C], f32, tag="sq")
                        nc.vector.tensor_tensor(out=sq[:, 0:outw], in0=orgap,
                                                in1=orgap, op=OP.mult)
                        nc.vector.tensor_tensor(out=accq[:, 0:outw],
                                                in0=accq[:, 0:outw],
                                                in1=sq[:, 0:outw], op=OP.add)

                    pend = []
                    for w in range(NW):
                        cw = CW[w]
                        off = int(woff[w])
                        attD = sbuf.tile([P, H], f32, tag="attD")
                        nc.sync.dma_start(
                            out=attD[:],
                            in_=loc[w * P : (w + 1) * P, C + H : C + 2 * H])
                        po = psumb.tile([P, C + 2 * H], f32, space="PSUM",
                                        tag="big")
                        nsub = (cw + CAP - 1) // CAP
                        szs = [cw // nsub + (1 if i < cw % nsub else 0)
                               for i in range(nsub)]
                        offs = [sum(szs[:i]) for i in range(nsub)]
                        for s in range(nsub):
                            c0 = offs[s]
                            ns = szs[s]
                            hg = gpool.tile([P, CAP, 2 * DW], f32, tag="hg")
                            nc.gpsimd.dma_gather(
                                out_ap=hg[:, 0:ns, :],
                                in_ap=tabv,
                                idxs_ap=idx_t[:, (off + c0) * 8 : (off + c0 + ns) * 8],
                                num_idxs=ns * P,
                                num_idxs_reg=ns * P,
                                elem_size=2 * DW,
                                single_packet=False,
                            )
                            hgv = hg[:, 0:ns, :].rearrange(
                                "p c (two d) -> p c two d", two=2)
                            ex = mpool.tile([P, CAP, 2, H], f32, tag="ex")
                            nc.vector.tensor_tensor(
                                out=ex[:, 0:ns],
                                in0=hgv[:, :, :, C : C + H],
                                in1=attD[:].unsqueeze(1).unsqueeze(1)
                                    .broadcast_to([P, ns, 2, H]),
                                op=OP.add)
                            lr = mpool.tile([P, CAP, 2, H], f32, tag="lr")
                            nc.vector.tensor_scalar(
                                out=lr[:, 0:ns], in0=ex[:, 0:ns], scalar1=0.2,
                                scalar2=None, op0=OP.mult)
                            nc.vector.tensor_tensor(
                                out=lr[:, 0:ns], in0=lr[:, 0:ns], in1=ex[:, 0:ns],
                                op=OP.max)
                            nc.scalar.activation(out=ex[:, 0:ns], in_=lr[:, 0:ns],
                                                 func=ACT.Exp)
                            nc.vector.tensor_tensor(
                                out=hgv[:, :, :, C : C + H],
                                in0=ex[:, 0:ns],
                                in1=pm_t[:, off + c0 : off + c0 + ns, :]
                                    .unsqueeze(3).broadcast_to([P, ns, 2, H]),
                                op=OP.mult)
                            for par in range(2):
                                nc.vector.tensor_tensor(
                                    out=hgv[:, :, par, 0:C].rearrange(
                                        "p c (k f) -> p c k f", k=H),
                                    in0=hgv[:, :, par, 0:C].rearrange(
                                        "p c (k f) -> p c k f", k=H),
                                    in1=hgv[:, :, par, C : C + H].unsqueeze(3)
                                        .broadcast_to([P, ns, H, F]),
                                    op=OP.mult)
                            for par in range(2):
                                for cc in range(ns):
                                    mm(po[:, 0 : C + H], ident[:],
                                       hgv[:, cc, par, 0 : C + H],
                                       start=(s == 0 and par == 0 and cc == 0),
                                       stop=(s == nsub - 1 and par == 1
                                             and cc == ns - 1))
                        pend.append((w, po))
                        if len(pend) == 2:
                            flush(*pend.pop(0))
                    for item in pend:
                        flush(*item)
                    mm(pstats[0:1, 0:outw], ones[:], accs[:, 0:outw],
                       start=True, stop=True)
                    mm(pstats2[0:1, 0:outw], ones[:], accq[:, 0:outw],
                       start=True, stop=True)
                    # moments -> AllReduce -> scale/shift rows in DRAM
                    stg0 = sbuf.tile([P, C], f32, tag="stg0")
                    nc.vector.tensor_copy(out=stg0[0:1, 0:outw],
                                          in_=pstats[0:1, 0:outw])
                    stg1 = sbuf.tile([P, C], f32, tag="stg1")
                    nc.vector.tensor_copy(out=stg1[0:1, 0:outw],
                                          in_=pstats2[0:1, 0:outw])
                    std = st1 if layer == 1 else st2
                    stdr = st1r if layer == 1 else st2r
                    nc.sync.dma_start(out=std[0:1, :], in_=stg0[0:1, 0:outw])
                    nc.sync.dma_start(out=std[1:2, :], in_=stg1[0:1, 0:outw])
                    if local_cc:
                        nc.sync.dma_start(out=stdr[:, :], in_=std[:])
                    else:
                        nc.gpsimd.collective_compute(
                            "AllReduce", OP.add, replica_groups=CCG,
                            ins=[std[:].opt()], outs=[stdr[:].opt()])
                    # single-partition workspace: slices share one partition
                    bn = cpool.tile([1, 10 * C], f32, tag="bn")
                    r0 = bn[:, 0 * C : 0 * C + outw]
                    r1 = bn[:, 1 * C : 1 * C + outw]
                    gv = bn[:, 2 * C : 2 * C + outw]
                    bev = bn[:, 3 * C : 3 * C + outw]
                    mu = bn[:, 4 * C : 4 * C + outw]
                    var = bn[:, 5 * C : 5 * C + outw]
                    msq = bn[:, 6 * C : 6 * C + outw]
                    rstd = bn[:, 7 * C : 7 * C + outw]
                    scl = bn[:, 8 * C : 8 * C + outw]
                    shf = bn[:, 9 * C : 9 * C + outw]
                    nc.sync.dma_start(out=r0, in_=stdr[0:1, :])
                    nc.sync.dma_start(out=r1, in_=stdr[1:2, :])
                    nc.sync.dma_start(out=gv, in_=(d_g1 if layer == 1 else d_g2)[:])
                    nc.sync.dma_start(out=bev, in_=(d_be1 if layer == 1 else d_be2)[:])
                    nc.vector.tensor_scalar(out=mu, in0=r0, scalar1=1.0 / N,
                                            scalar2=None, op0=OP.mult)
                    nc.vector.tensor_scalar(out=var, in0=r1, scalar1=1.0 / N,
                                            scalar2=None, op0=OP.mult)
                    nc.vector.tensor_tensor(out=msq, in0=mu, in1=mu, op=OP.mult)
                    nc.vector.tensor_tensor(out=var, in0=var, in1=msq, op=OP.subtract)
                    nc.vector.tensor_scalar(out=var, in0=var, scalar1=EPS,
                                            scalar2=None, op0=OP.add)
                    nc.scalar.activation(out=msq, in_=var, func=ACT.Sqrt)
                    nc.vector.reciprocal(out=rstd, in_=msq)
                    nc.vector.tensor_tensor(out=scl, in0=gv, in1=rstd, op=OP.mult)
                    nc.vector.tensor_tensor(out=shf, in0=mu, in1=scl, op=OP.mult)
                    nc.vector.tensor_tensor(out=shf, in0=bev, in1=shf, op=OP.subtract)
                    scd = sc1 if layer == 1 else sc2
                    nc.sync.dma_start(out=scd[0:1, :], in_=scl)
                    nc.sync.dma_start(out=scd[1:2, :], in_=shf)

                edge_phase(tab1, loc1, 1)

                # ---------------- phase E ------------------------------------
                sccol1 = sbuf.tile([P, 4], f32, tag="sccol1")
                nc.sync.dma_start(
                    out=sccol1[:].rearrange("p (r h) -> p r h", r=2),
                    in_=sc1[:].rearrange("r (h p) -> p r h", p=P))

                def phaseE_rows(t):
                    g1r = org1_all[:, t, :]
                    yT = []
                    for hf in range(2):
                        ptt = psum.tile([P, P], f32, space="PSUM", tag="tr")
                        nc.tensor.transpose(out=ptt[:],
                                            in_=g1r[:, hf * P : (hf + 1) * P],
                                            identity=ident[:])
                        yt = sbuf.tile([P, P], f32, tag=f"yT{hf}")
                        nc.vector.tensor_scalar(
                            out=yt[:], in0=ptt[:],
                            scalar1=sccol1[:, hf : hf + 1],
                            scalar2=sccol1[:, 2 + hf : 3 + hf],
                            op0=OP.mult, op1=OP.add)
                        nc.vector.tensor_scalar(out=yt[:], in0=yt[:], scalar1=0.0,
                                                scalar2=None, op0=OP.max)
                        yT.append(yt)
                    return yT

                build_table(phaseE_rows, W2_t, WA2_t, loc2)
                if local_cc:
                    nc.sync.dma_start(out=tab2[0:NPCP, :], in_=loc2[:])
                else:
                    nc.gpsimd.collective_compute(
                        "AllGather", OP.bypass, replica_groups=CCG,
                        ins=[loc2[:].opt()], outs=[tab2[:].opt()])

                edge_phase(tab2, loc2, 2)

                # ---------------- phase I ------------------------------------
                sccol2 = sbuf.tile([P, 2], f32, tag="sccol2")
                nc.sync.dma_start(out=sccol2[0:F, :],
                                  in_=sc2[:].rearrange("r f -> f r"))
                for t in range(NW):
                    g2r = org2_all[:, t, :]
                    ptt = psum.tile([P, P], f32, space="PSUM", tag="tr")
                    nc.tensor.transpose(out=ptt[0:F, :], in_=g2r,
                                        identity=ident[:])
                    y2T = sbuf.tile([P, P], f32, tag="y2T")
                    nc.vector.tensor_scalar(
                        out=y2T[0:F, :], in0=ptt[0:F, :],
                        scalar1=sccol2[0:F, 0:1], scalar2=sccol2[0:F, 1:2],
                        op0=OP.mult, op1=OP.add)
                    pzt = psumb.tile([P, C + 2 * H], f32, space="PSUM", tag="big")
                    pz = pzt[:, 0:END]
                    mm(pz, y2T[0:F, :], Wo1_t[0:F, :], start=True, stop=False)
                    mm(pz, ones_r2[0:1, :], bo1_t[0:1, :], start=False, stop=True)
                    zr = sbuf.tile([P, END], f32, tag="zr")
                    nc.vector.tensor_scalar(out=zr[:], in0=pz[:], scalar1=0.0,
                                            scalar2=None, op0=OP.max)
                    zw = sbuf.tile([P, C], f32, tag="zw")
                    nc.vector.tensor_tensor(out=zw[:], in0=zr[:], in1=Wo2r_t[:],
                                            op=OP.mult)
                    res = res_all[:, t : t + 1]
                    nc.vector.tensor_reduce(out=res, in_=zw[:], axis=AX.X,
                                            op=OP.add)
                    nc.vector.tensor_tensor(out=res, in0=res, in1=bo2r_t[:],
                                            op=OP.add)
                nc.sync.dma_start(
                    out=d_out[:].rearrange("(t p) o -> p (t o)", p=P),
                    in_=res_all[:])

    nc.compile()
    return nc


def kernel(**inputs):
    X = np.asarray(inputs["X"], np.float32)
    prep = _host_prep(X, inputs["edge_index"])
    wts = _build_weights(inputs)

    key = ("prog", tuple(prep["CW"]))
    if key not in _CACHE:
        _CACHE.clear()
        _CACHE[key] = _build_program(prep["CW"], prep["woff"], prep["slots"])
    nc = _CACHE[key]

    in_maps = []
    for c in range(NCORES):
        m = dict(
            xrows=prep["xrows"][c],
            idx_tiles=prep["idx_tiles"][c],
            pm=prep["pm"][c],
        )
        m.update(wts)
        in_maps.append(m)

    from concourse.bass_utils import run_bass_kernel_spmd
    res = run_bass_kernel_spmd(nc, in_maps, list(range(NCORES)))

    out = np.zeros((N, 1), np.float32)
    for c in range(NCORES):
        pc = prep["perm"][c * NPCP : (c + 1) * NPCP]
        m = pc >= 0
        out[pc[m]] = res.results[c]["out"][m, :]
    return out

